# revision 10
# baseline (speedup 1.0000x reference)
"""AudioGRU Trainium2 Bass kernel.

Single-layer GRU (PyTorch gate order r,z,n) over T=2000 steps followed by a
mean over time. Data-parallel over the batch axis across 8 NeuronCores
(B=256 -> 32 per core); weights replicated; the time recurrence is local.

Device kernel: everything lives transposed on-chip, [H=128 partitions,
batch free]. Per step the gate pre-activations gh = W_hh @ h accumulate into
PSUM on top of the input projections gx = W_ih_aug @ [x_t; 1] (the augmented
ones-row bakes the biases into PSUM), which a block "sweep" matmul computes
16 steps ahead using PE idle time. r and z live in one 2-bank PSUM tensor so
a single fused sigmoid covers both. With split_mm the recurrence matmul is
decomposed as W @ h = W @ (ncv + dd) with dd = z*h_prev and ncv = (1-z)*n,
so the h-update add leaves the critical cycle. The state h is bf16; x ships
as fp8_e4m3 (W_ih stays bf16 — mixed non-fp32 matmul dtypes are allowed),
which halves the host->device transfer; measured end-to-end rel err ~5e-3
vs the fp32 reference, within the 2e-2 gate.

Host path: the jax/PJRT executable is built once and cached at module level
(the generic run_bass_kernel_spmd re-traces and re-lowers on every call,
which costs ~5.5s/call); inputs are assembled directly into the global
sharded layout and the output is gathered with a single device->host fetch.
"""

import os
import sys
import numpy as np
import ml_dtypes
from contextlib import ExitStack

for _p in ("/opt/trn_rl_repo", "/root/.axon_site/_ro/trn_rl_repo"):
    if os.path.isdir(_p) and _p not in sys.path:
        sys.path.insert(0, _p)

B, T, I, H = 256, 2000, 23, 128
IA = I + 1                # augmented input rows (ones row carries biases)
NCORES = 8
BL = B // NCORES          # 32 batch per core
BLK = 16                  # psum block: 16 steps * 32 batch = 512 f32 = one bank
CHUNK = 400               # x DMA chunk + host transfer chunk, in timesteps
assert CHUNK % BLK == 0
bf16 = ml_dtypes.bfloat16
f8 = ml_dtypes.float8_e4m3

_PROG_CACHE = {}
OPTS = frozenset(("split_mm", "hsum_pool"))


def _emit(ctx, tc, nc, xTs, wih, bn, whh, yT, T_, repeat=1):
    from concourse import mybir

    f32, b16 = mybir.dt.float32, mybir.dt.bfloat16
    fp8 = mybir.dt.float8e4
    AF = mybir.ActivationFunctionType
    OP = mybir.AluOpType
    NBLK = T_ // BLK
    nchunk = (T_ + CHUNK - 1) // CHUNK
    split = "split_mm" in OPTS
    heng = nc.gpsimd if "hsum_pool" in OPTS else nc.vector

    const = ctx.enter_context(tc.tile_pool(name="const", bufs=1))
    xpool = ctx.enter_context(tc.tile_pool(name="xp", bufs=3))
    gxp_rz = ctx.enter_context(tc.tile_pool(name="gxrz", bufs=2, space="PSUM"))
    gxp_n = ctx.enter_context(tc.tile_pool(name="gxn", bufs=2, space="PSUM"))
    ghp = ctx.enter_context(tc.tile_pool(name="ghp", bufs=1, space="PSUM"))
    work = ctx.enter_context(tc.tile_pool(name="wk", bufs=3))

    wih_sb = const.tile([IA, 3 * H], b16, name="wih_sb")
    nc.sync.dma_start(wih_sb[:], wih)
    whh_sb = const.tile([H, 3 * H], b16, name="whh_sb")
    nc.sync.dma_start(whh_sb[:], whh)
    bn_sb = const.tile([H, 1], f32, name="bn_sb")
    nc.sync.dma_start(bn_sb[:], bn)

    h = const.tile([H, BL], b16, name="h_state")
    hs = const.tile([H, BL], f32, name="h_sum")

    if repeat > 1:
        ctx.enter_context(tc.For_i(0, repeat, 1, name="rep"))
    if not split:
        nc.vector.memset(h[:], 0.0)
    nc.vector.memset(hs[:], 0.0)

    xs = []

    def load_chunk(c):
        steps = min(CHUNK, T_ - c * CHUNK)
        xc = xpool.tile([IA, steps * BL], fp8, name="xc", tag="xc")
        nc.sync.dma_start(xc[:], xTs[c][:, :, :])
        return xc

    xs.append(load_chunk(0))
    if nchunk > 1:
        xs.append(load_chunk(1))

    # gh_n scratch bank: two rotating [H, BL] slots
    GHW = 512 if "ghn_2bank" in OPTS else BL
    ghn = ghp.tile([H, 1024 if "ghn_2bank" in OPTS else 512], f32, name="ghn_bank")

    def alloc_block():
        # r and z share one 2-bank tensor: cols 0..512 = r, 512..1024 = z.
        grz = gxp_rz.tile([H, 2 * BLK * BL], f32, name="grz", tag="grz")
        gn = gxp_n.tile([H, BLK * BL], f32, name="gn", tag="gn")
        gns = None
        if "gxn_sbuf" in OPTS:
            gns = work.tile([H, BLK * BL], f32, name="gns", tag="gns", bufs=2)
        return (grz, gn, gns)

    def sweep_block(blk, b):
        # Input projections (and biases, via the aug row) for block b.
        t0 = b * BLK
        c, o = divmod(t0, CHUNK)
        rhs = xs[c][:, o * BL : (o + BLK) * BL]
        grz, gn, gns = blk
        for g, out in ((0, grz[:, : BLK * BL]), (1, grz[:, BLK * BL :]), (2, gn[:, :])):
            nc.tensor.matmul(
                out,
                wih_sb[:, g * H : (g + 1) * H],
                rhs,
                start=True,
                stop=(g == 2),
                skip_group_check=True,
            )
        if gns is not None:
            nc.scalar.copy(gns[:], gn[:])

    blocks = [None, None]

    def rec_mms(vec, t_target, first, last):
        # Accumulate W_g @ vec into step t_target's gate psum slices.
        bt, jt = divmod(t_target, BLK)
        grz, gn = blocks[bt % 2][:2]
        slt = (t_target % 2) * GHW
        blk_last = last and jt == BLK - 1
        nc.tensor.matmul(
            grz[:, jt * BL : (jt + 1) * BL],
            whh_sb[:, 0:H], vec, start=False, stop=blk_last,
            skip_group_check=True,
        )
        nc.tensor.matmul(
            grz[:, BLK * BL + jt * BL : BLK * BL + (jt + 1) * BL],
            whh_sb[:, H : 2 * H], vec, start=False, stop=blk_last,
            skip_group_check=True,
        )
        nc.tensor.matmul(
            ghn[:, slt : slt + BL],
            whh_sb[:, 2 * H : 3 * H], vec, start=first, stop=last,
            skip_group_check=True,
        )

    blocks[0] = alloc_block()
    sweep_block(blocks[0], 0)

    ncv_p = None  # previous step's ncv (split mode)
    for t in range(T_):
        b_, j = divmod(t, BLK)
        if j == 0:
            if t % CHUNK == 0 and t // CHUNK + 2 < nchunk:
                xs.append(load_chunk(t // CHUNK + 2))
            if b_ + 1 < NBLK:
                blocks[(b_ + 1) % 2] = alloc_block()
                sweep_block(blocks[(b_ + 1) % 2], b_ + 1)

        grz, gn, gns = blocks[b_ % 2]
        sl = slice(j * BL, (j + 1) * BL)
        slz = slice(BLK * BL + j * BL, BLK * BL + (j + 1) * BL)
        slot = (t % 2) * GHW

        if "x_notdep" in OPTS:
            if t > 0:
                rec_mms(whh_sb[:, 0:BL], t, first=False, last=True)
        elif split:
            # Step t's gh accumulated from dd_{t-1} (emitted last step) plus
            # ncv_{t-1} here; nothing at t=0 (h_{-1} = 0).
            if ncv_p is not None:
                rec_mms(ncv_p[:], t, first=False, last=True)
        else:
            rec_mms(h[:], t, first=True, last=True)

        # Fused sigmoid over r|z (biases already in psum), then 1-z as a
        # sigmoid with scale=-1, placed between so tanh isn't delayed.
        rz = work.tile([H, 2, BL], f32, name="rz", tag="rz")
        if "unfuse_sig" in OPTS:
            nc.scalar.activation(rz[:, 0, :], grz[:, sl], AF.Sigmoid)
            nc.scalar.activation(rz[:, 1, :], grz[:, slz], AF.Sigmoid)
        else:
            nc.scalar.activation(
                rz[:],
                grz.rearrange("p (g c) -> p g c", g=2)[:, :, j * BL : (j + 1) * BL],
                AF.Sigmoid,
            )
        cc = work.tile([H, BL], f32, name="cc", tag="cc")
        if "cc_pool" in OPTS:
            heng.tensor_scalar(cc[:], rz[:, 1, :], -1.0, 1.0, OP.mult, OP.add)
        elif "cc_dve" in OPTS:
            nc.vector.tensor_scalar(cc[:], rz[:, 1, :], -1.0, 1.0, OP.mult, OP.add)
        else:
            nc.scalar.activation(cc[:], grz[:, slz], AF.Sigmoid, scale=-1.0)

        # t1 = (gh_n + b_hh_n) * r ; t2 = t1 + gx_n ; n = tanh(t2)
        t1 = work.tile([H, BL], f32, name="t1", tag="t1")
        if split and t == 0:
            nc.vector.tensor_scalar(t1[:], rz[:, 0, :], bn_sb[:, 0:1], None, OP.mult)
        else:
            nc.vector.scalar_tensor_tensor(
                t1[:], ghn[:, slot : slot + BL], bn_sb[:, 0:1], rz[:, 0, :],
                OP.add, OP.mult,
            )
        t2 = work.tile([H, BL], f32, name="t2", tag="t2")
        gn_src = gns if gns is not None else gn
        nc.vector.tensor_tensor(t2[:], t1[:], gn_src[:, sl], OP.add)
        nn = work.tile([H, BL], f32, name="nn", tag="nn")
        nc.scalar.activation(nn[:], t1[:] if "x_not2" in OPTS else t2[:], AF.Tanh)

        dd = work.tile([H, BL], b16 if split else f32, name="dd", tag="dd")
        if split and t == 0:
            nc.vector.tensor_scalar(dd[:], rz[:, 1, :], 0.0, None, OP.mult)
        else:
            nc.vector.tensor_tensor(dd[:], rz[:, 1, :], h[:], OP.mult)

        if split and t + 1 < T_:
            # dd's matmuls fire before tanh completes.
            rec_mms(whh_sb[:, 0:BL] if "x_notdep" in OPTS else dd[:], t + 1, first=True, last=False)

        ncv = work.tile([H, BL], b16 if split else f32, name="ncv", tag="ncv")
        nc.vector.tensor_tensor(ncv[:], nn[:], cc[:], OP.mult)

        if split and t + 1 < T_:
            pass  # ncv's matmuls are emitted at the top of step t+1

        # h = (1-z)n + z h, for the running sum and next step's dd.
        heng.tensor_tensor(h[:], ncv[:], dd[:], OP.add)
        heng.tensor_tensor(hs[:], hs[:], h[:], OP.add)

        ncv_p = ncv

    out_sb = const.tile([H, BL], f32, name="out_sb")
    nc.scalar.mul(out_sb[:], hs[:], 1.0 / T_)
    nc.sync.dma_start(yT, out_sb[:])


def build_program(T_=T, repeat=1):
    key = (T_, repeat, OPTS)
    if key in _PROG_CACHE:
        return _PROG_CACHE[key]
    import concourse.tile as tile
    from concourse import bacc, mybir

    f32, b16 = mybir.dt.float32, mybir.dt.bfloat16
    fp8 = mybir.dt.float8e4
    nc = bacc.Bacc(
        "TRN2", target_bir_lowering=False, debug=False, num_devices=NCORES
    )
    nchunk = (T_ + CHUNK - 1) // CHUNK
    xTs = [
        nc.dram_tensor(
            f"xT{c}",
            [IA, min(CHUNK, T_ - c * CHUNK), BL],
            fp8,
            kind="ExternalInput",
        ).ap()
        for c in range(nchunk)
    ]
    wih = nc.dram_tensor("wih", [IA, 3 * H], b16, kind="ExternalInput").ap()
    bn = nc.dram_tensor("bn", [H, 1], f32, kind="ExternalInput").ap()
    whh = nc.dram_tensor("whh", [H, 3 * H], b16, kind="ExternalInput").ap()
    yT = nc.dram_tensor("yT", [H, BL], f32, kind="ExternalOutput").ap()

    with tile.TileContext(nc) as tc:
        with ExitStack() as ctx:
            _emit(ctx, tc, nc, xTs, wih, bn, whh, yT, T_, repeat)
    nc.compile()
    _PROG_CACHE[key] = nc
    return nc


def _prep_weights(W_ih, W_hh, b_ih, b_hh):
    W_ih = np.asarray(W_ih, dtype=np.float32)
    W_hh = np.asarray(W_hh, dtype=np.float32)
    b_ih = np.asarray(b_ih, dtype=np.float32)
    b_hh = np.asarray(b_hh, dtype=np.float32)

    # Augmented input weights: last row carries the psum-resident biases
    # (b_r_tot, b_z_tot, b_ih_n).  b_hh_n is applied inside the r-multiply.
    wihT = np.concatenate([W_ih.T, np.zeros((1, 3 * H), np.float32)], axis=0)
    wihT[I, 0:H] = b_ih[0:H] + b_hh[0:H]
    wihT[I, H : 2 * H] = b_ih[H : 2 * H] + b_hh[H : 2 * H]
    wihT[I, 2 * H :] = b_ih[2 * H :]
    wihT = np.ascontiguousarray(wihT).astype(bf16)     # [IA, 3H]
    whhT = np.ascontiguousarray(W_hh.T).astype(bf16)   # [H, 3H]
    bnv = b_hh[2 * H :].reshape(H, 1).astype(np.float32)
    return wihT, whhT, bnv


def _prep_x_chunk(x, c, T_=T):
    # [B, T, I] f32 -> chunk c transposed fp8 with ones row:
    # (NC, IA, steps, BL) where element (n, i, t, b) = x[n*BL+b, c*CHUNK+t, i]
    steps = min(CHUNK, T_ - c * CHUNK)
    xs = x.reshape(NCORES, BL, x.shape[1], I)[:, :, c * CHUNK : c * CHUNK + steps]
    out = np.empty((NCORES, IA, steps, BL), dtype=f8)
    out[:, :I] = xs.transpose(0, 3, 2, 1).astype(f8)
    out[:, I] = np.float32(1.0)
    return out


def make_in_maps(x, W_ih, W_hh, b_ih, b_hh, T_=T):
    # Per-core input dicts (used by the traced bass_utils path).
    wihT, whhT, bnv = _prep_weights(W_ih, W_hh, b_ih, b_hh)
    x = np.asarray(x, dtype=np.float32)
    nchunk = (T_ + CHUNK - 1) // CHUNK
    chunks = [_prep_x_chunk(x, c, T_) for c in range(nchunk)]
    return [
        {
            **{f"xT{k}": chunks[k][c] for k in range(nchunk)},
            "wih": wihT,
            "whh": whhT,
            "bn": bnv,
        }
        for c in range(NCORES)
    ]


class _Runner:
    """Caches the built Bass program and the jitted sharded executable.

    run_bass_kernel_spmd constructs a fresh jax.jit(shard_map(...)) closure
    per call, which re-traces, re-lowers and re-invokes the NEFF compile
    hook every time (~5.5s/call measured). Building it once here makes the
    warm path pure transfer+execute.
    """

    def __init__(self, T_=T):
        import jax
        from jax.sharding import Mesh, PartitionSpec
        from jax.experimental.shard_map import shard_map as shard_map_fn
        from concourse import mybir
        from concourse.bass2jax import (
            _bass_exec_p,
            install_neuronx_cc_hook,
            partition_id_tensor,
        )

        install_neuronx_cc_hook()
        nc = build_program(T_)
        self.nc = nc
        self.T_ = T_

        partition_name = (
            nc.partition_id_tensor.name if nc.partition_id_tensor else None
        )
        in_names, out_names, out_avals = [], [], []
        for alloc in nc.m.functions[0].allocations:
            if not isinstance(alloc, mybir.MemoryLocationSet):
                continue
            name = alloc.memorylocations[0].name
            if alloc.kind == "ExternalInput":
                if name != partition_name:
                    in_names.append(name)
            elif alloc.kind == "ExternalOutput":
                shape = tuple(alloc.tensor_shape)
                dtype = mybir.dt.np(alloc.dtype)
                out_names.append(name)
                out_avals.append(jax.core.ShapedArray(shape, dtype))
        self.in_names = in_names
        self.out_names = out_names
        self.out_avals = out_avals
        n_params = len(in_names)
        n_outs = len(out_avals)
        in_names_all = in_names + out_names + (
            [partition_name] if partition_name else []
        )
        donate = tuple(range(n_params, n_params + n_outs))

        def _body(*args):
            operands = list(args)
            if partition_name:
                operands.append(partition_id_tensor())
            outs = _bass_exec_p.bind(
                *operands,
                out_avals=tuple(out_avals),
                in_names=tuple(in_names_all),
                out_names=tuple(out_names),
                lowering_input_output_aliases=(),
                sim_require_finite=True,
                sim_require_nnan=True,
                nc=nc,
            )
            return tuple(outs)

        from jax.sharding import NamedSharding

        devices = jax.devices()[:NCORES]
        assert len(devices) == NCORES
        mesh = Mesh(np.asarray(devices), ("core",))
        self.x_sharding = NamedSharding(mesh, PartitionSpec("core"))
        self._device_put = jax.device_put
        self.sharded = jax.jit(
            shard_map_fn(
                _body,
                mesh=mesh,
                in_specs=(PartitionSpec("core"),) * (n_params + n_outs),
                out_specs=(PartitionSpec("core"),) * n_outs,
                check_rep=False,
            ),
            donate_argnums=donate,
            keep_unused=True,
        )
        self.nchunk = (T_ + CHUNK - 1) // CHUNK

    def __call__(self, x, W_ih, W_hh, b_ih, b_hh):
        # Prep and ship x chunk by chunk: device_put is async, so chunk k's
        # host-side transpose+fp8 cast overlaps chunk k-1's transfer.
        x = np.asarray(x, dtype=np.float32)
        by_name = {}
        for c in range(self.nchunk):
            xc = _prep_x_chunk(x, c, self.T_)
            by_name[f"xT{c}"] = self._device_put(
                xc.reshape(NCORES * IA, xc.shape[2], BL), self.x_sharding
            )
        wihT, whhT, bnv = _prep_weights(W_ih, W_hh, b_ih, b_hh)
        by_name["wih"] = np.ascontiguousarray(
            np.broadcast_to(wihT, (NCORES, IA, 3 * H))
        ).reshape(NCORES * IA, 3 * H)
        by_name["whh"] = np.ascontiguousarray(
            np.broadcast_to(whhT, (NCORES, H, 3 * H))
        ).reshape(NCORES * H, 3 * H)
        by_name["bn"] = np.ascontiguousarray(
            np.broadcast_to(bnv, (NCORES, H, 1))
        ).reshape(NCORES * H, 1)
        concat_in = [by_name[n] for n in self.in_names]
        concat_zeros = [
            np.zeros((NCORES * a.shape[0], *a.shape[1:]), a.dtype)
            for a in self.out_avals
        ]
        out = self.sharded(*concat_in, *concat_zeros)
        # yT is [H, BL] per core -> global (NCORES*H, BL); fetch the 8
        # shards concurrently (each fetch is one RPC to its device).
        yT_arr = out[self.out_names.index("yT")]
        shards = yT_arr.addressable_shards
        import concurrent.futures as _cf

        with _cf.ThreadPoolExecutor(len(shards)) as ex:
            datas = list(ex.map(lambda s: np.asarray(s.data), shards))
        yT = np.empty((NCORES * H, BL), np.float32)
        for s, d in zip(shards, datas):
            yT[s.index] = d
        y = yT.reshape(NCORES, H, BL).transpose(0, 2, 1).reshape(B, H)
        return np.ascontiguousarray(y, dtype=np.float32)


_RUNNER = None


def _get_runner():
    global _RUNNER
    if _RUNNER is None:
        _RUNNER = _Runner()
    return _RUNNER


def run(x, W_ih, W_hh, b_ih, b_hh, T_=T, trace=False, **kw):
    if trace:
        # Traced path (NTFF profile) via the generic SPMD runner; raises in
        # environments without the axon NTFF hook.
        from concourse import bass_utils

        nc = build_program(T_)
        in_maps = make_in_maps(x, W_ih, W_hh, b_ih, b_hh, T_)
        res = bass_utils.run_bass_kernel_spmd(
            nc, in_maps, core_ids=list(range(NCORES)), trace=True, **kw
        )
        y = np.concatenate(
            [np.asarray(r["yT"], dtype=np.float32).T for r in res.results], axis=0
        )
        return y, res

    runner = _get_runner() if T_ == T else _Runner(T_)
    y = runner(x, W_ih, W_hh, b_ih, b_hh)

    class _Res:
        exec_time_ns = None
        results = None

    return y, _Res()


def kernel(**inputs) -> np.ndarray:
    runner = _get_runner()
    return runner(
        inputs["x"], inputs["W_ih"], inputs["W_hh"], inputs["b_ih"], inputs["b_hh"]
    )


# revision 26
# speedup vs baseline: 1.1001x; 1.1001x over previous
"""AudioGRU Trainium2 Bass kernel.

Single-layer GRU (PyTorch gate order r,z,n) over T=2000 steps followed by a
mean over time. Data-parallel over the batch axis across 8 NeuronCores
(B=256 -> 32 per core); weights replicated; the time recurrence is local.

Device kernel: everything lives transposed on-chip, [H=128 partitions,
batch free]. Per step the gate pre-activations gh = W_hh @ h accumulate into
PSUM on top of the input projections gx = W_ih @ x_t (computed by a block
"sweep" matmul 16 steps ahead using PE idle time); the r/z/n biases land in
the same PSUM banks via a K=1 matmul of the bias row against a const ones
tile. r and z live in one 2-bank PSUM tensor so a single fused sigmoid
covers both. With split_mm the recurrence matmul is decomposed as
W @ h = W @ (ncv + dd) with dd = z*h_prev and ncv = (1-z)*n, so the h-update
add leaves the critical cycle. The state h is bf16; x ships as fp8_e4m3
(W_ih stays bf16 — mixed non-fp32 matmul dtypes are allowed), which halves
the host->device transfer; measured end-to-end rel err ~5e-3 vs the fp32
reference, within the 2e-2 gate. A trailing device-side AllGather replicates
the [H, BL] result across cores so the host fetches one shard.

Host path (the wall-clock bottleneck — the tunnel to the device moves
~50 MB/s): the jax/PJRT executable is built once and cached at module level
(the generic run_bass_kernel_spmd re-traces and re-lowers on every call,
which costs ~5.5s/call). x is cast/transposed per T-chunk with torch (4x
faster than ml_dtypes) and each chunk is device_put asynchronously so prep
overlaps transfer; output buffers from the previous call are re-donated to
skip the host->device zero-init transfer.
"""

import os
import sys
import numpy as np
import ml_dtypes
from contextlib import ExitStack

for _p in ("/opt/trn_rl_repo", "/root/.axon_site/_ro/trn_rl_repo"):
    if os.path.isdir(_p) and _p not in sys.path:
        sys.path.insert(0, _p)

B, T, I, H = 256, 2000, 23, 128
IA = I + 1                # augmented input rows (ones row carries biases)
NCORES = 8
BL = B // NCORES          # 32 batch per core
BLK = 16                  # psum block: 16 steps * 32 batch = 512 f32 = one bank
CHUNK = 400               # x DMA chunk + host transfer chunk, in timesteps
assert CHUNK % BLK == 0
bf16 = ml_dtypes.bfloat16
f8 = ml_dtypes.float8_e4m3

_PROG_CACHE = {}
OPTS = frozenset(("split_mm", "hsum_pool"))


def _emit(ctx, tc, nc, xTs, wih, wb, bn, whh, yT, T_, repeat=1):
    from concourse import mybir

    f32, b16 = mybir.dt.float32, mybir.dt.bfloat16
    fp8 = mybir.dt.float8e4
    AF = mybir.ActivationFunctionType
    OP = mybir.AluOpType
    NBLK = T_ // BLK
    nchunk = (T_ + CHUNK - 1) // CHUNK
    split = "split_mm" in OPTS
    heng = nc.gpsimd if "hsum_pool" in OPTS else nc.vector

    const = ctx.enter_context(tc.tile_pool(name="const", bufs=1))
    xpool = ctx.enter_context(tc.tile_pool(name="xp", bufs=3))
    gxp_rz = ctx.enter_context(tc.tile_pool(name="gxrz", bufs=2, space="PSUM"))
    gxp_n = ctx.enter_context(tc.tile_pool(name="gxn", bufs=2, space="PSUM"))
    ghp = ctx.enter_context(tc.tile_pool(name="ghp", bufs=1, space="PSUM"))
    work = ctx.enter_context(tc.tile_pool(name="wk", bufs=3))

    wih_sb = const.tile([I, 3 * H], b16, name="wih_sb")
    nc.sync.dma_start(wih_sb[:], wih)
    wb_sb = const.tile([1, 3 * H], b16, name="wb_sb")
    nc.sync.dma_start(wb_sb[:], wb)
    whh_sb = const.tile([H, 3 * H], b16, name="whh_sb")
    nc.sync.dma_start(whh_sb[:], whh)
    bn_sb = const.tile([H, 1], f32, name="bn_sb")
    nc.sync.dma_start(bn_sb[:], bn)
    # K=1 matmuls against this ones row add the (r,z,n) biases into PSUM,
    # replacing the augmented ones-row that used to ship with x.
    ones_sb = const.tile([1, BLK * BL], b16, name="ones_sb")
    nc.vector.memset(ones_sb[:], 1.0)

    h = const.tile([H, BL], b16, name="h_state")
    hs = const.tile([H, BL], f32, name="h_sum")

    if repeat > 1:
        ctx.enter_context(tc.For_i(0, repeat, 1, name="rep"))
    if not split:
        nc.vector.memset(h[:], 0.0)
    nc.vector.memset(hs[:], 0.0)

    xs = []

    def load_chunk(c):
        steps = min(CHUNK, T_ - c * CHUNK)
        xc = xpool.tile([I, steps * BL], fp8, name="xc", tag="xc")
        nc.sync.dma_start(xc[:], xTs[c][:, :, :])
        return xc

    xs.append(load_chunk(0))
    if nchunk > 1:
        xs.append(load_chunk(1))

    # gh_n scratch bank: two rotating [H, BL] slots
    GHW = 512 if "ghn_2bank" in OPTS else BL
    ghn = ghp.tile([H, 1024 if "ghn_2bank" in OPTS else 512], f32, name="ghn_bank")

    def alloc_block():
        # r and z share one 2-bank tensor: cols 0..512 = r, 512..1024 = z.
        grz = gxp_rz.tile([H, 2 * BLK * BL], f32, name="grz", tag="grz")
        gn = gxp_n.tile([H, BLK * BL], f32, name="gn", tag="gn")
        gns = None
        if "gxn_sbuf" in OPTS:
            gns = work.tile([H, BLK * BL], f32, name="gns", tag="gns", bufs=2)
        return (grz, gn, gns)

    def sweep_block(blk, b):
        # Input projections plus biases (K=1 matmul on the ones row) for
        # block b.
        t0 = b * BLK
        c, o = divmod(t0, CHUNK)
        rhs = xs[c][:, o * BL : (o + BLK) * BL]
        grz, gn, gns = blk
        for g, out in ((0, grz[:, : BLK * BL]), (1, grz[:, BLK * BL :]), (2, gn[:, :])):
            nc.tensor.matmul(
                out,
                wih_sb[:, g * H : (g + 1) * H],
                rhs,
                start=True,
                stop=False,
                skip_group_check=True,
            )
            nc.tensor.matmul(
                out,
                wb_sb[:, g * H : (g + 1) * H],
                ones_sb[:],
                start=False,
                stop=(g == 2),
                skip_group_check=True,
            )
        if gns is not None:
            nc.scalar.copy(gns[:], gn[:])

    blocks = [None, None]

    def rec_mms(vec, t_target, first, last):
        # Accumulate W_g @ vec into step t_target's gate psum slices.
        bt, jt = divmod(t_target, BLK)
        grz, gn = blocks[bt % 2][:2]
        slt = (t_target % 2) * GHW
        blk_last = last and jt == BLK - 1
        nc.tensor.matmul(
            grz[:, jt * BL : (jt + 1) * BL],
            whh_sb[:, 0:H], vec, start=False, stop=blk_last,
            skip_group_check=True,
        )
        nc.tensor.matmul(
            grz[:, BLK * BL + jt * BL : BLK * BL + (jt + 1) * BL],
            whh_sb[:, H : 2 * H], vec, start=False, stop=blk_last,
            skip_group_check=True,
        )
        nc.tensor.matmul(
            ghn[:, slt : slt + BL],
            whh_sb[:, 2 * H : 3 * H], vec, start=first, stop=last,
            skip_group_check=True,
        )

    blocks[0] = alloc_block()
    sweep_block(blocks[0], 0)

    ncv_p = None  # previous step's ncv (split mode)
    for t in range(T_):
        b_, j = divmod(t, BLK)
        if j == 0:
            if t % CHUNK == 0 and t // CHUNK + 2 < nchunk:
                xs.append(load_chunk(t // CHUNK + 2))
            if b_ + 1 < NBLK:
                blocks[(b_ + 1) % 2] = alloc_block()
                sweep_block(blocks[(b_ + 1) % 2], b_ + 1)

        grz, gn, gns = blocks[b_ % 2]
        sl = slice(j * BL, (j + 1) * BL)
        slz = slice(BLK * BL + j * BL, BLK * BL + (j + 1) * BL)
        slot = (t % 2) * GHW

        if "x_notdep" in OPTS:
            if t > 0:
                rec_mms(whh_sb[:, 0:BL], t, first=False, last=True)
        elif split:
            # Step t's gh accumulated from dd_{t-1} (emitted last step) plus
            # ncv_{t-1} here; nothing at t=0 (h_{-1} = 0).
            if ncv_p is not None:
                rec_mms(ncv_p[:], t, first=False, last=True)
        else:
            rec_mms(h[:], t, first=True, last=True)

        # Fused sigmoid over r|z (biases already in psum), then 1-z as a
        # sigmoid with scale=-1, placed between so tanh isn't delayed.
        rz = work.tile([H, 2, BL], f32, name="rz", tag="rz")
        if "unfuse_sig" in OPTS:
            nc.scalar.activation(rz[:, 0, :], grz[:, sl], AF.Sigmoid)
            nc.scalar.activation(rz[:, 1, :], grz[:, slz], AF.Sigmoid)
        else:
            nc.scalar.activation(
                rz[:],
                grz.rearrange("p (g c) -> p g c", g=2)[:, :, j * BL : (j + 1) * BL],
                AF.Sigmoid,
            )
        cc = work.tile([H, BL], f32, name="cc", tag="cc")
        if "cc_pool" in OPTS:
            heng.tensor_scalar(cc[:], rz[:, 1, :], -1.0, 1.0, OP.mult, OP.add)
        elif "cc_dve" in OPTS:
            nc.vector.tensor_scalar(cc[:], rz[:, 1, :], -1.0, 1.0, OP.mult, OP.add)
        else:
            nc.scalar.activation(cc[:], grz[:, slz], AF.Sigmoid, scale=-1.0)

        # t1 = (gh_n + b_hh_n) * r ; t2 = t1 + gx_n ; n = tanh(t2)
        t1 = work.tile([H, BL], f32, name="t1", tag="t1")
        if split and t == 0:
            nc.vector.tensor_scalar(t1[:], rz[:, 0, :], bn_sb[:, 0:1], None, OP.mult)
        else:
            nc.vector.scalar_tensor_tensor(
                t1[:], ghn[:, slot : slot + BL], bn_sb[:, 0:1], rz[:, 0, :],
                OP.add, OP.mult,
            )
        t2 = work.tile([H, BL], f32, name="t2", tag="t2")
        gn_src = gns if gns is not None else gn
        nc.vector.tensor_tensor(t2[:], t1[:], gn_src[:, sl], OP.add)
        nn = work.tile([H, BL], f32, name="nn", tag="nn")
        nc.scalar.activation(nn[:], t1[:] if "x_not2" in OPTS else t2[:], AF.Tanh)

        dd = work.tile([H, BL], b16 if split else f32, name="dd", tag="dd")
        if split and t == 0:
            nc.vector.tensor_scalar(dd[:], rz[:, 1, :], 0.0, None, OP.mult)
        else:
            nc.vector.tensor_tensor(dd[:], rz[:, 1, :], h[:], OP.mult)

        if split and t + 1 < T_:
            # dd's matmuls fire before tanh completes.
            rec_mms(whh_sb[:, 0:BL] if "x_notdep" in OPTS else dd[:], t + 1, first=True, last=False)

        ncv = work.tile([H, BL], b16 if split else f32, name="ncv", tag="ncv")
        nc.vector.tensor_tensor(ncv[:], nn[:], cc[:], OP.mult)

        if split and t + 1 < T_:
            pass  # ncv's matmuls are emitted at the top of step t+1

        # h = (1-z)n + z h, for the running sum and next step's dd.
        heng.tensor_tensor(h[:], ncv[:], dd[:], OP.add)
        heng.tensor_tensor(hs[:], hs[:], h[:], OP.add)

        ncv_p = ncv

    out_sb = const.tile([H, BL], f32, name="out_sb")
    nc.scalar.mul(out_sb[:], hs[:], 1.0 / T_)
    # Device-side AllGather so every core's yT holds all 8 cores' results;
    # the host then fetches a single shard (1 RPC instead of 8).
    dram = ctx.enter_context(tc.tile_pool(name="ydram", bufs=1, space="DRAM"))
    y_in = dram.tile([H, BL], f32, name="y_in")
    y_out = dram.tile([NCORES * H, BL], f32, name="y_out")
    nc.sync.dma_start(y_in[:], out_sb[:])
    nc.gpsimd.collective_compute(
        "AllGather",
        mybir.AluOpType.bypass,
        replica_groups=[list(range(NCORES))],
        ins=[y_in.opt()],
        outs=[y_out.opt()],
    )
    nc.sync.dma_start(yT, y_out[:])


def build_program(T_=T, repeat=1):
    key = (T_, repeat, OPTS)
    if key in _PROG_CACHE:
        return _PROG_CACHE[key]
    import concourse.tile as tile
    from concourse import bacc, mybir

    f32, b16 = mybir.dt.float32, mybir.dt.bfloat16
    fp8 = mybir.dt.float8e4
    nc = bacc.Bacc(
        "TRN2", target_bir_lowering=False, debug=False, num_devices=NCORES
    )
    nchunk = (T_ + CHUNK - 1) // CHUNK
    xTs = [
        nc.dram_tensor(
            f"xT{c}",
            [I, min(CHUNK, T_ - c * CHUNK), BL],
            fp8,
            kind="ExternalInput",
        ).ap()
        for c in range(nchunk)
    ]
    wih = nc.dram_tensor("wih", [I, 3 * H], b16, kind="ExternalInput").ap()
    wb = nc.dram_tensor("wb", [1, 3 * H], b16, kind="ExternalInput").ap()
    bn = nc.dram_tensor("bn", [H, 1], f32, kind="ExternalInput").ap()
    whh = nc.dram_tensor("whh", [H, 3 * H], b16, kind="ExternalInput").ap()
    yT = nc.dram_tensor("yT", [NCORES * H, BL], f32, kind="ExternalOutput").ap()

    with tile.TileContext(nc) as tc:
        with ExitStack() as ctx:
            _emit(ctx, tc, nc, xTs, wih, wb, bn, whh, yT, T_, repeat)
    nc.compile()
    _PROG_CACHE[key] = nc
    return nc


def _prep_weights(W_ih, W_hh, b_ih, b_hh):
    W_ih = np.asarray(W_ih, dtype=np.float32)
    W_hh = np.asarray(W_hh, dtype=np.float32)
    b_ih = np.asarray(b_ih, dtype=np.float32)
    b_hh = np.asarray(b_hh, dtype=np.float32)

    # Bias row, applied in PSUM via a K=1 matmul against an on-device ones
    # row: (b_r_tot, b_z_tot, b_ih_n).  b_hh_n is applied inside the
    # r-multiply (bn).
    wbr = np.empty((1, 3 * H), np.float32)
    wbr[0, 0:H] = b_ih[0:H] + b_hh[0:H]
    wbr[0, H : 2 * H] = b_ih[H : 2 * H] + b_hh[H : 2 * H]
    wbr[0, 2 * H :] = b_ih[2 * H :]
    wbr = wbr.astype(bf16)                             # [1, 3H]
    wihT = np.ascontiguousarray(W_ih.T).astype(bf16)   # [I, 3H]
    whhT = np.ascontiguousarray(W_hh.T).astype(bf16)   # [H, 3H]
    bnv = b_hh[2 * H :].reshape(H, 1).astype(np.float32)
    return wihT, wbr, whhT, bnv


try:
    import torch as _torch

    _TORCH_F8 = _torch.float8_e4m3fn  # bit-identical to ml_dtypes.float8_e4m3
except Exception:
    _torch = None


def _prep_x_chunk(x, c, T_=T):
    # [B, T, I] f32 -> chunk c transposed fp8 (no ones row; filled on-device):
    # (NC, I, steps, BL) where element (n, i, t, b) = x[n*BL+b, c*CHUNK+t, i]
    steps = min(CHUNK, T_ - c * CHUNK)
    if _torch is not None:
        t = _torch.from_numpy(x).reshape(NCORES, BL, x.shape[1], I)[
            :, :, c * CHUNK : c * CHUNK + steps
        ].permute(0, 3, 2, 1)
        out = _torch.empty((NCORES, I, steps, BL), dtype=_TORCH_F8)
        out.copy_(t)
        return out.view(_torch.uint8).numpy().view(f8)
    xs = x.reshape(NCORES, BL, x.shape[1], I)[:, :, c * CHUNK : c * CHUNK + steps]
    return xs.transpose(0, 3, 2, 1).astype(f8)


def make_in_maps(x, W_ih, W_hh, b_ih, b_hh, T_=T):
    # Per-core input dicts (used by the traced bass_utils path).
    wihT, wbr, whhT, bnv = _prep_weights(W_ih, W_hh, b_ih, b_hh)
    x = np.asarray(x, dtype=np.float32)
    nchunk = (T_ + CHUNK - 1) // CHUNK
    chunks = [_prep_x_chunk(x, c, T_) for c in range(nchunk)]
    return [
        {
            **{f"xT{k}": chunks[k][c] for k in range(nchunk)},
            "wih": wihT,
            "wb": wbr,
            "whh": whhT,
            "bn": bnv,
        }
        for c in range(NCORES)
    ]


class _Runner:
    """Caches the built Bass program and the jitted sharded executable.

    run_bass_kernel_spmd constructs a fresh jax.jit(shard_map(...)) closure
    per call, which re-traces, re-lowers and re-invokes the NEFF compile
    hook every time (~5.5s/call measured). Building it once here makes the
    warm path pure transfer+execute.
    """

    def __init__(self, T_=T):
        import jax
        from jax.sharding import Mesh, PartitionSpec
        from jax.experimental.shard_map import shard_map as shard_map_fn
        from concourse import mybir
        from concourse.bass2jax import (
            _bass_exec_p,
            install_neuronx_cc_hook,
            partition_id_tensor,
        )

        install_neuronx_cc_hook()
        nc = build_program(T_)
        self.nc = nc
        self.T_ = T_

        partition_name = (
            nc.partition_id_tensor.name if nc.partition_id_tensor else None
        )
        in_names, out_names, out_avals = [], [], []
        for alloc in nc.m.functions[0].allocations:
            if not isinstance(alloc, mybir.MemoryLocationSet):
                continue
            name = alloc.memorylocations[0].name
            if alloc.kind == "ExternalInput":
                if name != partition_name:
                    in_names.append(name)
            elif alloc.kind == "ExternalOutput":
                shape = tuple(alloc.tensor_shape)
                dtype = mybir.dt.np(alloc.dtype)
                out_names.append(name)
                out_avals.append(jax.core.ShapedArray(shape, dtype))
        self.in_names = in_names
        self.out_names = out_names
        self.out_avals = out_avals
        n_params = len(in_names)
        n_outs = len(out_avals)
        in_names_all = in_names + out_names + (
            [partition_name] if partition_name else []
        )
        donate = tuple(range(n_params, n_params + n_outs))

        def _body(*args):
            operands = list(args)
            if partition_name:
                operands.append(partition_id_tensor())
            outs = _bass_exec_p.bind(
                *operands,
                out_avals=tuple(out_avals),
                in_names=tuple(in_names_all),
                out_names=tuple(out_names),
                lowering_input_output_aliases=(),
                sim_require_finite=True,
                sim_require_nnan=True,
                nc=nc,
            )
            return tuple(outs)

        from jax.sharding import NamedSharding

        devices = jax.devices()[:NCORES]
        assert len(devices) == NCORES
        mesh = Mesh(np.asarray(devices), ("core",))
        self.x_sharding = NamedSharding(mesh, PartitionSpec("core"))
        self._device_put = jax.device_put
        self.sharded = jax.jit(
            shard_map_fn(
                _body,
                mesh=mesh,
                in_specs=(PartitionSpec("core"),) * (n_params + n_outs),
                out_specs=(PartitionSpec("core"),) * n_outs,
                check_rep=False,
            ),
            donate_argnums=donate,
            keep_unused=True,
        )
        self.nchunk = (T_ + CHUNK - 1) // CHUNK
        self._last_out = None  # previous call's output buffers, re-donated

    def __call__(self, x, W_ih, W_hh, b_ih, b_hh):
        # Prep and ship x chunk by chunk: device_put is async, so chunk k's
        # host-side transpose+fp8 cast overlaps chunk k-1's transfer.
        x = np.asarray(x, dtype=np.float32)
        by_name = {}
        for c in range(self.nchunk):
            xc = _prep_x_chunk(x, c, self.T_)
            by_name[f"xT{c}"] = self._device_put(
                xc.reshape(NCORES * I, xc.shape[2], BL), self.x_sharding
            )
        wihT, wbr, whhT, bnv = _prep_weights(W_ih, W_hh, b_ih, b_hh)
        by_name["wih"] = np.ascontiguousarray(
            np.broadcast_to(wihT, (NCORES, I, 3 * H))
        ).reshape(NCORES * I, 3 * H)
        by_name["wb"] = np.ascontiguousarray(
            np.broadcast_to(wbr, (NCORES, 1, 3 * H))
        ).reshape(NCORES * 1, 3 * H)
        by_name["whh"] = np.ascontiguousarray(
            np.broadcast_to(whhT, (NCORES, H, 3 * H))
        ).reshape(NCORES * H, 3 * H)
        by_name["bn"] = np.ascontiguousarray(
            np.broadcast_to(bnv, (NCORES, H, 1))
        ).reshape(NCORES * H, 1)
        concat_in = [by_name[n] for n in self.in_names]
        # The kernel writes every output element, so the donated output
        # buffers' contents are irrelevant; re-donate the previous call's
        # device-resident outputs to skip the host->device zero transfer.
        if self._last_out is not None:
            donation = self._last_out
        else:
            donation = [
                np.zeros((NCORES * a.shape[0], *a.shape[1:]), a.dtype)
                for a in self.out_avals
            ]
        out = self.sharded(*concat_in, *donation)
        self._last_out = list(out)
        # yT is allgathered on-device: every core's output holds all 8
        # cores' results -> fetch a single shard (1 RPC instead of 8).
        yT_arr = out[self.out_names.index("yT")]
        shard0 = min(yT_arr.addressable_shards, key=lambda s: s.index[0].start or 0)
        yT = np.asarray(shard0.data)
        y = yT.reshape(NCORES, H, BL).transpose(0, 2, 1).reshape(B, H)
        return np.ascontiguousarray(y, dtype=np.float32)


_RUNNER = None


def _get_runner():
    global _RUNNER
    if _RUNNER is None:
        _RUNNER = _Runner()
    return _RUNNER


def run(x, W_ih, W_hh, b_ih, b_hh, T_=T, trace=False, **kw):
    if trace:
        # Traced path (NTFF profile) via the generic SPMD runner; raises in
        # environments without the axon NTFF hook.
        from concourse import bass_utils

        nc = build_program(T_)
        in_maps = make_in_maps(x, W_ih, W_hh, b_ih, b_hh, T_)
        res = bass_utils.run_bass_kernel_spmd(
            nc, in_maps, core_ids=list(range(NCORES)), trace=True, **kw
        )
        yT = np.asarray(res.results[0]["yT"], dtype=np.float32)
        y = yT.reshape(NCORES, H, BL).transpose(0, 2, 1).reshape(B, H)
        return y, res

    runner = _get_runner() if T_ == T else _Runner(T_)
    y = runner(x, W_ih, W_hh, b_ih, b_hh)

    class _Res:
        exec_time_ns = None
        results = None

    return y, _Res()


def kernel(**inputs) -> np.ndarray:
    runner = _get_runner()
    return runner(
        inputs["x"], inputs["W_ih"], inputs["W_hh"], inputs["b_ih"], inputs["b_hh"]
    )


# revision 28
# speedup vs baseline: 1.1138x; 1.0124x over previous
"""AudioGRU Trainium2 Bass kernel.

Single-layer GRU (PyTorch gate order r,z,n) over T=2000 steps followed by a
mean over time. Data-parallel over the batch axis across 8 NeuronCores
(B=256 -> 32 per core); weights replicated; the time recurrence is local.

Device kernel: everything lives transposed on-chip, [H=128 partitions,
batch free]. Per step the gate pre-activations gh = W_hh @ h accumulate into
PSUM on top of the input projections gx = W_ih @ x_t (computed by a block
"sweep" matmul 16 steps ahead using PE idle time); the r/z/n biases land in
the same PSUM banks via a K=1 matmul of the bias row against a const ones
tile. r and z live in one 2-bank PSUM tensor so a single fused sigmoid
covers both. With split_mm the recurrence matmul is decomposed as
W @ h = W @ (ncv + dd) with dd = z*h_prev and ncv = (1-z)*n, so the h-update
add leaves the critical cycle. The state h is bf16; x ships as fp8_e4m3
(W_ih stays bf16 — mixed non-fp32 matmul dtypes are allowed), which halves
the host->device transfer; measured end-to-end rel err ~5e-3 vs the fp32
reference, within the 2e-2 gate. A trailing device-side AllGather replicates
the [H, BL] result across cores so the host fetches one shard.

Host path (the wall-clock bottleneck — the tunnel to the device moves
~50 MB/s): the jax/PJRT executable is built once and cached at module level
(the generic run_bass_kernel_spmd re-traces and re-lowers on every call,
which costs ~5.5s/call). x is cast/transposed per T-chunk with torch (4x
faster than ml_dtypes) and each chunk is device_put asynchronously so prep
overlaps transfer; output buffers from the previous call are re-donated to
skip the host->device zero-init transfer.
"""

import os
import sys
import numpy as np
import ml_dtypes
from contextlib import ExitStack

for _p in ("/opt/trn_rl_repo", "/root/.axon_site/_ro/trn_rl_repo"):
    if os.path.isdir(_p) and _p not in sys.path:
        sys.path.insert(0, _p)

# Persistent XLA executable cache: a fresh process reuses the compiled
# (NEFF-embedding) executable instead of re-running the ~1 min compile.
os.environ.setdefault("JAX_COMPILATION_CACHE_DIR", "/tmp/jax_exe_cache")
os.environ.setdefault("JAX_PERSISTENT_CACHE_MIN_COMPILE_TIME_SECS", "0")
os.environ.setdefault("JAX_PERSISTENT_CACHE_MIN_ENTRY_SIZE_BYTES", "0")

B, T, I, H = 256, 2000, 23, 128
IA = I + 1                # augmented input rows (ones row carries biases)
NCORES = 8
BL = B // NCORES          # 32 batch per core
BLK = 16                  # psum block: 16 steps * 32 batch = 512 f32 = one bank
CHUNK = 400               # x DMA chunk + host transfer chunk, in timesteps
assert CHUNK % BLK == 0
bf16 = ml_dtypes.bfloat16
f8 = ml_dtypes.float8_e4m3

_PROG_CACHE = {}
OPTS = frozenset(("split_mm", "hsum_pool"))


def _emit(ctx, tc, nc, xTs, wih, wb, bn, whh, yT, T_, repeat=1):
    from concourse import mybir

    f32, b16 = mybir.dt.float32, mybir.dt.bfloat16
    fp8 = mybir.dt.float8e4
    AF = mybir.ActivationFunctionType
    OP = mybir.AluOpType
    NBLK = T_ // BLK
    nchunk = (T_ + CHUNK - 1) // CHUNK
    split = "split_mm" in OPTS
    heng = nc.gpsimd if "hsum_pool" in OPTS else nc.vector

    const = ctx.enter_context(tc.tile_pool(name="const", bufs=1))
    xpool = ctx.enter_context(tc.tile_pool(name="xp", bufs=3))
    gxp_rz = ctx.enter_context(tc.tile_pool(name="gxrz", bufs=2, space="PSUM"))
    gxp_n = ctx.enter_context(tc.tile_pool(name="gxn", bufs=2, space="PSUM"))
    ghp = ctx.enter_context(tc.tile_pool(name="ghp", bufs=1, space="PSUM"))
    work = ctx.enter_context(tc.tile_pool(name="wk", bufs=3))

    wih_sb = const.tile([I, 3 * H], b16, name="wih_sb")
    nc.sync.dma_start(wih_sb[:], wih)
    wb_sb = const.tile([1, 3 * H], b16, name="wb_sb")
    nc.sync.dma_start(wb_sb[:], wb)
    whh_sb = const.tile([H, 3 * H], b16, name="whh_sb")
    nc.sync.dma_start(whh_sb[:], whh)
    bn_sb = const.tile([H, 1], f32, name="bn_sb")
    nc.sync.dma_start(bn_sb[:], bn)
    # K=1 matmuls against this ones row add the (r,z,n) biases into PSUM,
    # replacing the augmented ones-row that used to ship with x.
    ones_sb = const.tile([1, BLK * BL], b16, name="ones_sb")
    nc.vector.memset(ones_sb[:], 1.0)

    h = const.tile([H, BL], b16, name="h_state")
    hs = const.tile([H, BL], f32, name="h_sum")

    if repeat > 1:
        ctx.enter_context(tc.For_i(0, repeat, 1, name="rep"))
    if not split:
        nc.vector.memset(h[:], 0.0)
    nc.vector.memset(hs[:], 0.0)

    xs = []

    def load_chunk(c):
        steps = min(CHUNK, T_ - c * CHUNK)
        xc = xpool.tile([I, steps * BL], fp8, name="xc", tag="xc")
        nc.sync.dma_start(xc[:], xTs[c][:, :, :])
        return xc

    xs.append(load_chunk(0))
    if nchunk > 1:
        xs.append(load_chunk(1))

    # gh_n scratch bank: two rotating [H, BL] slots
    GHW = 512 if "ghn_2bank" in OPTS else BL
    ghn = ghp.tile([H, 1024 if "ghn_2bank" in OPTS else 512], f32, name="ghn_bank")

    def alloc_block():
        # r and z share one 2-bank tensor: cols 0..512 = r, 512..1024 = z.
        grz = gxp_rz.tile([H, 2 * BLK * BL], f32, name="grz", tag="grz")
        gn = gxp_n.tile([H, BLK * BL], f32, name="gn", tag="gn")
        gns = None
        if "gxn_sbuf" in OPTS:
            gns = work.tile([H, BLK * BL], f32, name="gns", tag="gns", bufs=2)
        return (grz, gn, gns)

    def sweep_block(blk, b):
        # Input projections plus biases (K=1 matmul on the ones row) for
        # block b.
        t0 = b * BLK
        c, o = divmod(t0, CHUNK)
        rhs = xs[c][:, o * BL : (o + BLK) * BL]
        grz, gn, gns = blk
        for g, out in ((0, grz[:, : BLK * BL]), (1, grz[:, BLK * BL :]), (2, gn[:, :])):
            nc.tensor.matmul(
                out,
                wih_sb[:, g * H : (g + 1) * H],
                rhs,
                start=True,
                stop=False,
                skip_group_check=True,
            )
            nc.tensor.matmul(
                out,
                wb_sb[:, g * H : (g + 1) * H],
                ones_sb[:],
                start=False,
                stop=(g == 2),
                skip_group_check=True,
            )
        if gns is not None:
            nc.scalar.copy(gns[:], gn[:])

    blocks = [None, None]

    def rec_mms(vec, t_target, first, last):
        # Accumulate W_g @ vec into step t_target's gate psum slices.
        bt, jt = divmod(t_target, BLK)
        grz, gn = blocks[bt % 2][:2]
        slt = (t_target % 2) * GHW
        blk_last = last and jt == BLK - 1
        nc.tensor.matmul(
            grz[:, jt * BL : (jt + 1) * BL],
            whh_sb[:, 0:H], vec, start=False, stop=blk_last,
            skip_group_check=True,
        )
        nc.tensor.matmul(
            grz[:, BLK * BL + jt * BL : BLK * BL + (jt + 1) * BL],
            whh_sb[:, H : 2 * H], vec, start=False, stop=blk_last,
            skip_group_check=True,
        )
        nc.tensor.matmul(
            ghn[:, slt : slt + BL],
            whh_sb[:, 2 * H : 3 * H], vec, start=first, stop=last,
            skip_group_check=True,
        )

    blocks[0] = alloc_block()
    sweep_block(blocks[0], 0)

    ncv_p = None  # previous step's ncv (split mode)
    for t in range(T_):
        b_, j = divmod(t, BLK)
        if j == 0:
            if t % CHUNK == 0 and t // CHUNK + 2 < nchunk:
                xs.append(load_chunk(t // CHUNK + 2))
            if b_ + 1 < NBLK:
                blocks[(b_ + 1) % 2] = alloc_block()
                sweep_block(blocks[(b_ + 1) % 2], b_ + 1)

        grz, gn, gns = blocks[b_ % 2]
        sl = slice(j * BL, (j + 1) * BL)
        slz = slice(BLK * BL + j * BL, BLK * BL + (j + 1) * BL)
        slot = (t % 2) * GHW

        if "x_notdep" in OPTS:
            if t > 0:
                rec_mms(whh_sb[:, 0:BL], t, first=False, last=True)
        elif split:
            # Step t's gh accumulated from dd_{t-1} (emitted last step) plus
            # ncv_{t-1} here; nothing at t=0 (h_{-1} = 0).
            if ncv_p is not None:
                rec_mms(ncv_p[:], t, first=False, last=True)
        else:
            rec_mms(h[:], t, first=True, last=True)

        # Fused sigmoid over r|z (biases already in psum), then 1-z as a
        # sigmoid with scale=-1, placed between so tanh isn't delayed.
        rz = work.tile([H, 2, BL], f32, name="rz", tag="rz")
        if "unfuse_sig" in OPTS:
            nc.scalar.activation(rz[:, 0, :], grz[:, sl], AF.Sigmoid)
            nc.scalar.activation(rz[:, 1, :], grz[:, slz], AF.Sigmoid)
        else:
            nc.scalar.activation(
                rz[:],
                grz.rearrange("p (g c) -> p g c", g=2)[:, :, j * BL : (j + 1) * BL],
                AF.Sigmoid,
            )
        cc = work.tile([H, BL], f32, name="cc", tag="cc")
        if "cc_pool" in OPTS:
            heng.tensor_scalar(cc[:], rz[:, 1, :], -1.0, 1.0, OP.mult, OP.add)
        elif "cc_dve" in OPTS:
            nc.vector.tensor_scalar(cc[:], rz[:, 1, :], -1.0, 1.0, OP.mult, OP.add)
        else:
            nc.scalar.activation(cc[:], grz[:, slz], AF.Sigmoid, scale=-1.0)

        # t1 = (gh_n + b_hh_n) * r ; t2 = t1 + gx_n ; n = tanh(t2)
        t1 = work.tile([H, BL], f32, name="t1", tag="t1")
        if split and t == 0:
            nc.vector.tensor_scalar(t1[:], rz[:, 0, :], bn_sb[:, 0:1], None, OP.mult)
        else:
            nc.vector.scalar_tensor_tensor(
                t1[:], ghn[:, slot : slot + BL], bn_sb[:, 0:1], rz[:, 0, :],
                OP.add, OP.mult,
            )
        t2 = work.tile([H, BL], f32, name="t2", tag="t2")
        gn_src = gns if gns is not None else gn
        nc.vector.tensor_tensor(t2[:], t1[:], gn_src[:, sl], OP.add)
        nn = work.tile([H, BL], f32, name="nn", tag="nn")
        nc.scalar.activation(nn[:], t1[:] if "x_not2" in OPTS else t2[:], AF.Tanh)

        dd = work.tile([H, BL], b16 if split else f32, name="dd", tag="dd")
        if split and t == 0:
            nc.vector.tensor_scalar(dd[:], rz[:, 1, :], 0.0, None, OP.mult)
        else:
            nc.vector.tensor_tensor(dd[:], rz[:, 1, :], h[:], OP.mult)

        if split and t + 1 < T_:
            # dd's matmuls fire before tanh completes.
            rec_mms(whh_sb[:, 0:BL] if "x_notdep" in OPTS else dd[:], t + 1, first=True, last=False)

        ncv = work.tile([H, BL], b16 if split else f32, name="ncv", tag="ncv")
        nc.vector.tensor_tensor(ncv[:], nn[:], cc[:], OP.mult)

        if split and t + 1 < T_:
            pass  # ncv's matmuls are emitted at the top of step t+1

        # h = (1-z)n + z h, for the running sum and next step's dd.
        heng.tensor_tensor(h[:], ncv[:], dd[:], OP.add)
        heng.tensor_tensor(hs[:], hs[:], h[:], OP.add)

        ncv_p = ncv

    out_sb = const.tile([H, BL], f32, name="out_sb")
    nc.scalar.mul(out_sb[:], hs[:], 1.0 / T_)
    # Device-side AllGather so every core's yT holds all 8 cores' results;
    # the host then fetches a single shard (1 RPC instead of 8).
    dram = ctx.enter_context(tc.tile_pool(name="ydram", bufs=1, space="DRAM"))
    y_in = dram.tile([H, BL], f32, name="y_in")
    y_out = dram.tile([NCORES * H, BL], f32, name="y_out")
    nc.sync.dma_start(y_in[:], out_sb[:])
    nc.gpsimd.collective_compute(
        "AllGather",
        mybir.AluOpType.bypass,
        replica_groups=[list(range(NCORES))],
        ins=[y_in.opt()],
        outs=[y_out.opt()],
    )
    nc.sync.dma_start(yT, y_out[:])


def build_program(T_=T, repeat=1):
    key = (T_, repeat, OPTS)
    if key in _PROG_CACHE:
        return _PROG_CACHE[key]
    import concourse.tile as tile
    from concourse import bacc, mybir

    f32, b16 = mybir.dt.float32, mybir.dt.bfloat16
    fp8 = mybir.dt.float8e4
    nc = bacc.Bacc(
        "TRN2", target_bir_lowering=False, debug=False, num_devices=NCORES
    )
    nchunk = (T_ + CHUNK - 1) // CHUNK
    xTs = [
        nc.dram_tensor(
            f"xT{c}",
            [I, min(CHUNK, T_ - c * CHUNK), BL],
            fp8,
            kind="ExternalInput",
        ).ap()
        for c in range(nchunk)
    ]
    wih = nc.dram_tensor("wih", [I, 3 * H], b16, kind="ExternalInput").ap()
    wb = nc.dram_tensor("wb", [1, 3 * H], b16, kind="ExternalInput").ap()
    bn = nc.dram_tensor("bn", [H, 1], f32, kind="ExternalInput").ap()
    whh = nc.dram_tensor("whh", [H, 3 * H], b16, kind="ExternalInput").ap()
    yT = nc.dram_tensor("yT", [NCORES * H, BL], f32, kind="ExternalOutput").ap()

    with tile.TileContext(nc) as tc:
        with ExitStack() as ctx:
            _emit(ctx, tc, nc, xTs, wih, wb, bn, whh, yT, T_, repeat)
    nc.compile()
    _PROG_CACHE[key] = nc
    return nc


def _prep_weights(W_ih, W_hh, b_ih, b_hh):
    W_ih = np.asarray(W_ih, dtype=np.float32)
    W_hh = np.asarray(W_hh, dtype=np.float32)
    b_ih = np.asarray(b_ih, dtype=np.float32)
    b_hh = np.asarray(b_hh, dtype=np.float32)

    # Bias row, applied in PSUM via a K=1 matmul against an on-device ones
    # row: (b_r_tot, b_z_tot, b_ih_n).  b_hh_n is applied inside the
    # r-multiply (bn).
    wbr = np.empty((1, 3 * H), np.float32)
    wbr[0, 0:H] = b_ih[0:H] + b_hh[0:H]
    wbr[0, H : 2 * H] = b_ih[H : 2 * H] + b_hh[H : 2 * H]
    wbr[0, 2 * H :] = b_ih[2 * H :]
    wbr = wbr.astype(bf16)                             # [1, 3H]
    wihT = np.ascontiguousarray(W_ih.T).astype(bf16)   # [I, 3H]
    whhT = np.ascontiguousarray(W_hh.T).astype(bf16)   # [H, 3H]
    bnv = b_hh[2 * H :].reshape(H, 1).astype(np.float32)
    return wihT, wbr, whhT, bnv


try:
    import torch as _torch

    _TORCH_F8 = _torch.float8_e4m3fn  # bit-identical to ml_dtypes.float8_e4m3
except Exception:
    _torch = None


def _prep_x_chunk(x, c, T_=T):
    # [B, T, I] f32 -> chunk c transposed fp8 (no ones row; filled on-device):
    # (NC, I, steps, BL) where element (n, i, t, b) = x[n*BL+b, c*CHUNK+t, i]
    steps = min(CHUNK, T_ - c * CHUNK)
    if _torch is not None:
        t = _torch.from_numpy(x).reshape(NCORES, BL, x.shape[1], I)[
            :, :, c * CHUNK : c * CHUNK + steps
        ].permute(0, 3, 2, 1)
        out = _torch.empty((NCORES, I, steps, BL), dtype=_TORCH_F8)
        out.copy_(t)
        return out.view(_torch.uint8).numpy().view(f8)
    xs = x.reshape(NCORES, BL, x.shape[1], I)[:, :, c * CHUNK : c * CHUNK + steps]
    return xs.transpose(0, 3, 2, 1).astype(f8)


def make_in_maps(x, W_ih, W_hh, b_ih, b_hh, T_=T):
    # Per-core input dicts (used by the traced bass_utils path).
    wihT, wbr, whhT, bnv = _prep_weights(W_ih, W_hh, b_ih, b_hh)
    x = np.asarray(x, dtype=np.float32)
    nchunk = (T_ + CHUNK - 1) // CHUNK
    chunks = [_prep_x_chunk(x, c, T_) for c in range(nchunk)]
    return [
        {
            **{f"xT{k}": chunks[k][c] for k in range(nchunk)},
            "wih": wihT,
            "wb": wbr,
            "whh": whhT,
            "bn": bnv,
        }
        for c in range(NCORES)
    ]


class _Runner:
    """Caches the built Bass program and the jitted sharded executable.

    run_bass_kernel_spmd constructs a fresh jax.jit(shard_map(...)) closure
    per call, which re-traces, re-lowers and re-invokes the NEFF compile
    hook every time (~5.5s/call measured). Building it once here makes the
    warm path pure transfer+execute.
    """

    def __init__(self, T_=T):
        import jax
        from jax.sharding import Mesh, PartitionSpec
        from jax.experimental.shard_map import shard_map as shard_map_fn
        from concourse import mybir
        from concourse.bass2jax import (
            _bass_exec_p,
            install_neuronx_cc_hook,
            partition_id_tensor,
        )

        install_neuronx_cc_hook()
        nc = build_program(T_)
        self.nc = nc
        self.T_ = T_

        partition_name = (
            nc.partition_id_tensor.name if nc.partition_id_tensor else None
        )
        in_names, out_names, out_avals = [], [], []
        for alloc in nc.m.functions[0].allocations:
            if not isinstance(alloc, mybir.MemoryLocationSet):
                continue
            name = alloc.memorylocations[0].name
            if alloc.kind == "ExternalInput":
                if name != partition_name:
                    in_names.append(name)
            elif alloc.kind == "ExternalOutput":
                shape = tuple(alloc.tensor_shape)
                dtype = mybir.dt.np(alloc.dtype)
                out_names.append(name)
                out_avals.append(jax.core.ShapedArray(shape, dtype))
        self.in_names = in_names
        self.out_names = out_names
        self.out_avals = out_avals
        n_params = len(in_names)
        n_outs = len(out_avals)
        in_names_all = in_names + out_names + (
            [partition_name] if partition_name else []
        )
        donate = tuple(range(n_params, n_params + n_outs))

        def _body(*args):
            operands = list(args)
            if partition_name:
                operands.append(partition_id_tensor())
            outs = _bass_exec_p.bind(
                *operands,
                out_avals=tuple(out_avals),
                in_names=tuple(in_names_all),
                out_names=tuple(out_names),
                lowering_input_output_aliases=(),
                sim_require_finite=True,
                sim_require_nnan=True,
                nc=nc,
            )
            return tuple(outs)

        from jax.sharding import NamedSharding

        devices = jax.devices()[:NCORES]
        assert len(devices) == NCORES
        mesh = Mesh(np.asarray(devices), ("core",))
        self.x_sharding = NamedSharding(mesh, PartitionSpec("core"))
        self._device_put = jax.device_put
        self.sharded = jax.jit(
            shard_map_fn(
                _body,
                mesh=mesh,
                in_specs=(PartitionSpec("core"),) * (n_params + n_outs),
                out_specs=(PartitionSpec("core"),) * n_outs,
                check_rep=False,
            ),
            donate_argnums=donate,
            keep_unused=True,
        )
        self.nchunk = (T_ + CHUNK - 1) // CHUNK
        self._last_out = None  # previous call's output buffers, re-donated

    def __call__(self, x, W_ih, W_hh, b_ih, b_hh):
        # Prep and ship x chunk by chunk: device_put is async, so chunk k's
        # host-side transpose+fp8 cast overlaps chunk k-1's transfer.
        x = np.asarray(x, dtype=np.float32)
        by_name = {}
        for c in range(self.nchunk):
            xc = _prep_x_chunk(x, c, self.T_)
            by_name[f"xT{c}"] = self._device_put(
                xc.reshape(NCORES * I, xc.shape[2], BL), self.x_sharding
            )
        wihT, wbr, whhT, bnv = _prep_weights(W_ih, W_hh, b_ih, b_hh)
        by_name["wih"] = np.ascontiguousarray(
            np.broadcast_to(wihT, (NCORES, I, 3 * H))
        ).reshape(NCORES * I, 3 * H)
        by_name["wb"] = np.ascontiguousarray(
            np.broadcast_to(wbr, (NCORES, 1, 3 * H))
        ).reshape(NCORES * 1, 3 * H)
        by_name["whh"] = np.ascontiguousarray(
            np.broadcast_to(whhT, (NCORES, H, 3 * H))
        ).reshape(NCORES * H, 3 * H)
        by_name["bn"] = np.ascontiguousarray(
            np.broadcast_to(bnv, (NCORES, H, 1))
        ).reshape(NCORES * H, 1)
        concat_in = [by_name[n] for n in self.in_names]
        # The kernel writes every output element, so the donated output
        # buffers' contents are irrelevant; re-donate the previous call's
        # device-resident outputs to skip the host->device zero transfer.
        # The first call device_puts zeros so every call presents the same
        # committed-array signature (a numpy-vs-device mismatch would
        # trigger a second executable compile).
        if self._last_out is not None:
            donation = self._last_out
        else:
            donation = [
                self._device_put(
                    np.zeros((NCORES * a.shape[0], *a.shape[1:]), a.dtype),
                    self.x_sharding,
                )
                for a in self.out_avals
            ]
        out = self.sharded(*concat_in, *donation)
        self._last_out = list(out)
        # yT is allgathered on-device: every core's output holds all 8
        # cores' results -> fetch a single shard (1 RPC instead of 8).
        yT_arr = out[self.out_names.index("yT")]
        shard0 = min(yT_arr.addressable_shards, key=lambda s: s.index[0].start or 0)
        yT = np.asarray(shard0.data)
        y = yT.reshape(NCORES, H, BL).transpose(0, 2, 1).reshape(B, H)
        return np.ascontiguousarray(y, dtype=np.float32)


_RUNNER = None


def _get_runner():
    global _RUNNER
    if _RUNNER is None:
        _RUNNER = _Runner()
    return _RUNNER


def run(x, W_ih, W_hh, b_ih, b_hh, T_=T, trace=False, **kw):
    if trace:
        # Traced path (NTFF profile) via the generic SPMD runner; raises in
        # environments without the axon NTFF hook.
        from concourse import bass_utils

        nc = build_program(T_)
        in_maps = make_in_maps(x, W_ih, W_hh, b_ih, b_hh, T_)
        res = bass_utils.run_bass_kernel_spmd(
            nc, in_maps, core_ids=list(range(NCORES)), trace=True, **kw
        )
        yT = np.asarray(res.results[0]["yT"], dtype=np.float32)
        y = yT.reshape(NCORES, H, BL).transpose(0, 2, 1).reshape(B, H)
        return y, res

    runner = _get_runner() if T_ == T else _Runner(T_)
    y = runner(x, W_ih, W_hh, b_ih, b_hh)

    class _Res:
        exec_time_ns = None
        results = None

    return y, _Res()


def kernel(**inputs) -> np.ndarray:
    runner = _get_runner()
    return runner(
        inputs["x"], inputs["W_ih"], inputs["W_hh"], inputs["b_ih"], inputs["b_hh"]
    )


# revision 32
# speedup vs baseline: 1.2404x; 1.1136x over previous
"""AudioGRU Trainium2 Bass kernel.

Single-layer GRU (PyTorch gate order r,z,n) over T=2000 steps followed by a
mean over time. Data-parallel over the batch axis across 8 NeuronCores
(B=256 -> 32 per core); weights replicated; the time recurrence is local.

Device kernel: everything lives transposed on-chip, [H=128 partitions,
batch free]. Per step the gate pre-activations gh = W_hh @ h accumulate into
PSUM on top of the input projections gx = W_ih @ x_t (computed by a block
"sweep" matmul 16 steps ahead using PE idle time); the r/z/n biases land in
the same PSUM banks via a K=1 matmul of the bias row against a const ones
tile. r and z live in one 2-bank PSUM tensor so a single fused sigmoid
covers both. With split_mm the recurrence matmul is decomposed as
W @ h = W @ (ncv + dd) with dd = z*h_prev and ncv = (1-z)*n, so the h-update
add leaves the critical cycle. The state h is bf16; x ships as fp8_e4m3
(W_ih stays bf16 — mixed non-fp32 matmul dtypes are allowed), which halves
the host->device transfer; measured end-to-end rel err ~5e-3 vs the fp32
reference, within the 2e-2 gate. A trailing device-side AllGather replicates
the [H, BL] result across cores so the host fetches one shard.

Host path (the wall-clock bottleneck — the tunnel to the device moves
~50 MB/s): the jax/PJRT executable is built once and cached at module level
(the generic run_bass_kernel_spmd re-traces and re-lowers on every call,
which costs ~5.5s/call). x is cast/transposed per T-chunk with torch (4x
faster than ml_dtypes) and each chunk is device_put asynchronously so prep
overlaps transfer; output buffers from the previous call are re-donated to
skip the host->device zero-init transfer.
"""

import os
import sys
import numpy as np
import ml_dtypes
from contextlib import ExitStack

for _p in ("/opt/trn_rl_repo", "/root/.axon_site/_ro/trn_rl_repo"):
    if os.path.isdir(_p) and _p not in sys.path:
        sys.path.insert(0, _p)



B, T, I, H = 256, 2000, 23, 128
NCORES = 8
BL = B // NCORES          # 32 batch per core
BLK = 16                  # psum block: 16 steps * 32 batch = 512 f32 = one bank
CHUNK = 400               # x DMA chunk + host transfer chunk, in timesteps
assert CHUNK % BLK == 0
bf16 = ml_dtypes.bfloat16
f8 = ml_dtypes.float8_e4m3

_PROG_CACHE = {}
OPTS = frozenset(("split_mm", "hsum_pool"))


def _emit(ctx, tc, nc, xTs, wih, wb, bn, whh, yT, T_, repeat=1):
    from concourse import mybir

    f32, b16 = mybir.dt.float32, mybir.dt.bfloat16
    fp8 = mybir.dt.float8e4
    AF = mybir.ActivationFunctionType
    OP = mybir.AluOpType
    NBLK = T_ // BLK
    nchunk = (T_ + CHUNK - 1) // CHUNK
    split = "split_mm" in OPTS
    heng = nc.gpsimd if "hsum_pool" in OPTS else nc.vector

    const = ctx.enter_context(tc.tile_pool(name="const", bufs=1))
    xpool = ctx.enter_context(tc.tile_pool(name="xp", bufs=3))
    gxp_rz = ctx.enter_context(tc.tile_pool(name="gxrz", bufs=2, space="PSUM"))
    gxp_n = ctx.enter_context(tc.tile_pool(name="gxn", bufs=2, space="PSUM"))
    ghp = ctx.enter_context(tc.tile_pool(name="ghp", bufs=1, space="PSUM"))
    work = ctx.enter_context(tc.tile_pool(name="wk", bufs=3))

    wih_sb = const.tile([I, 3 * H], b16, name="wih_sb")
    nc.sync.dma_start(wih_sb[:], wih)
    wb_sb = const.tile([1, 3 * H], b16, name="wb_sb")
    nc.sync.dma_start(wb_sb[:], wb)
    whh_sb = const.tile([H, 3 * H], b16, name="whh_sb")
    nc.sync.dma_start(whh_sb[:], whh)
    bn_sb = const.tile([H, 1], f32, name="bn_sb")
    nc.sync.dma_start(bn_sb[:], bn)
    # K=1 matmuls against this ones row add the (r,z,n) biases into PSUM,
    # replacing the augmented ones-row that used to ship with x.
    ones_sb = const.tile([1, BLK * BL], b16, name="ones_sb")
    nc.vector.memset(ones_sb[:], 1.0)

    h = const.tile([H, BL], b16, name="h_state")
    hs = const.tile([H, BL], f32, name="h_sum")

    if repeat > 1:
        ctx.enter_context(tc.For_i(0, repeat, 1, name="rep"))
    if not split:
        nc.vector.memset(h[:], 0.0)
    nc.vector.memset(hs[:], 0.0)

    xs = []

    def load_chunk(c):
        steps = min(CHUNK, T_ - c * CHUNK)
        xc = xpool.tile([I, steps * BL], fp8, name="xc", tag="xc")
        nc.sync.dma_start(xc[:], xTs[c][:, :, :])
        return xc

    xs.append(load_chunk(0))
    if nchunk > 1:
        xs.append(load_chunk(1))

    # gh_n scratch bank: two rotating [H, BL] slots
    GHW = 512 if "ghn_2bank" in OPTS else BL
    ghn = ghp.tile([H, 1024 if "ghn_2bank" in OPTS else 512], f32, name="ghn_bank")

    def alloc_block():
        # r and z share one 2-bank tensor: cols 0..512 = r, 512..1024 = z.
        grz = gxp_rz.tile([H, 2 * BLK * BL], f32, name="grz", tag="grz")
        gn = gxp_n.tile([H, BLK * BL], f32, name="gn", tag="gn")
        gns = None
        if "gxn_sbuf" in OPTS:
            gns = work.tile([H, BLK * BL], f32, name="gns", tag="gns", bufs=2)
        return (grz, gn, gns)

    def sweep_block(blk, b):
        # Input projections plus biases (K=1 matmul on the ones row) for
        # block b.
        t0 = b * BLK
        c, o = divmod(t0, CHUNK)
        rhs = xs[c][:, o * BL : (o + BLK) * BL]
        grz, gn, gns = blk
        for g, out in ((0, grz[:, : BLK * BL]), (1, grz[:, BLK * BL :]), (2, gn[:, :])):
            nc.tensor.matmul(
                out,
                wih_sb[:, g * H : (g + 1) * H],
                rhs,
                start=True,
                stop=False,
                skip_group_check=True,
            )
            nc.tensor.matmul(
                out,
                wb_sb[:, g * H : (g + 1) * H],
                ones_sb[:],
                start=False,
                stop=(g == 2),
                skip_group_check=True,
            )
        if gns is not None:
            nc.scalar.copy(gns[:], gn[:])

    blocks = [None, None]

    def rec_mms(vec, t_target, first, last):
        # Accumulate W_g @ vec into step t_target's gate psum slices.
        bt, jt = divmod(t_target, BLK)
        grz, gn = blocks[bt % 2][:2]
        slt = (t_target % 2) * GHW
        blk_last = last and jt == BLK - 1
        nc.tensor.matmul(
            grz[:, jt * BL : (jt + 1) * BL],
            whh_sb[:, 0:H], vec, start=False, stop=blk_last,
            skip_group_check=True,
        )
        nc.tensor.matmul(
            grz[:, BLK * BL + jt * BL : BLK * BL + (jt + 1) * BL],
            whh_sb[:, H : 2 * H], vec, start=False, stop=blk_last,
            skip_group_check=True,
        )
        nc.tensor.matmul(
            ghn[:, slt : slt + BL],
            whh_sb[:, 2 * H : 3 * H], vec, start=first, stop=last,
            skip_group_check=True,
        )

    blocks[0] = alloc_block()
    sweep_block(blocks[0], 0)

    ncv_p = None  # previous step's ncv (split mode)
    for t in range(T_):
        b_, j = divmod(t, BLK)
        if j == 0:
            if t % CHUNK == 0 and t // CHUNK + 2 < nchunk:
                xs.append(load_chunk(t // CHUNK + 2))
            if b_ + 1 < NBLK:
                blocks[(b_ + 1) % 2] = alloc_block()
                sweep_block(blocks[(b_ + 1) % 2], b_ + 1)

        grz, gn, gns = blocks[b_ % 2]
        sl = slice(j * BL, (j + 1) * BL)
        slz = slice(BLK * BL + j * BL, BLK * BL + (j + 1) * BL)
        slot = (t % 2) * GHW

        if "x_notdep" in OPTS:
            if t > 0:
                rec_mms(whh_sb[:, 0:BL], t, first=False, last=True)
        elif split:
            # Step t's gh accumulated from dd_{t-1} (emitted last step) plus
            # ncv_{t-1} here; nothing at t=0 (h_{-1} = 0).
            if ncv_p is not None:
                rec_mms(ncv_p[:], t, first=False, last=True)
        else:
            rec_mms(h[:], t, first=True, last=True)

        # Fused sigmoid over r|z (biases already in psum), then 1-z as a
        # sigmoid with scale=-1, placed between so tanh isn't delayed.
        rz = work.tile([H, 2, BL], f32, name="rz", tag="rz")
        if "unfuse_sig" in OPTS:
            nc.scalar.activation(rz[:, 0, :], grz[:, sl], AF.Sigmoid)
            nc.scalar.activation(rz[:, 1, :], grz[:, slz], AF.Sigmoid)
        else:
            nc.scalar.activation(
                rz[:],
                grz.rearrange("p (g c) -> p g c", g=2)[:, :, j * BL : (j + 1) * BL],
                AF.Sigmoid,
            )
        cc = work.tile([H, BL], f32, name="cc", tag="cc")
        if "cc_pool" in OPTS:
            heng.tensor_scalar(cc[:], rz[:, 1, :], -1.0, 1.0, OP.mult, OP.add)
        elif "cc_dve" in OPTS:
            nc.vector.tensor_scalar(cc[:], rz[:, 1, :], -1.0, 1.0, OP.mult, OP.add)
        else:
            nc.scalar.activation(cc[:], grz[:, slz], AF.Sigmoid, scale=-1.0)

        # t1 = (gh_n + b_hh_n) * r ; t2 = t1 + gx_n ; n = tanh(t2)
        t1 = work.tile([H, BL], f32, name="t1", tag="t1")
        if split and t == 0:
            nc.vector.tensor_scalar(t1[:], rz[:, 0, :], bn_sb[:, 0:1], None, OP.mult)
        else:
            nc.vector.scalar_tensor_tensor(
                t1[:], ghn[:, slot : slot + BL], bn_sb[:, 0:1], rz[:, 0, :],
                OP.add, OP.mult,
            )
        t2 = work.tile([H, BL], f32, name="t2", tag="t2")
        gn_src = gns if gns is not None else gn
        nc.vector.tensor_tensor(t2[:], t1[:], gn_src[:, sl], OP.add)
        nn = work.tile([H, BL], f32, name="nn", tag="nn")
        nc.scalar.activation(nn[:], t1[:] if "x_not2" in OPTS else t2[:], AF.Tanh)

        dd = work.tile([H, BL], b16 if split else f32, name="dd", tag="dd")
        if split and t == 0:
            nc.vector.tensor_scalar(dd[:], rz[:, 1, :], 0.0, None, OP.mult)
        else:
            nc.vector.tensor_tensor(dd[:], rz[:, 1, :], h[:], OP.mult)

        if split and t + 1 < T_:
            # dd's matmuls fire before tanh completes.
            rec_mms(whh_sb[:, 0:BL] if "x_notdep" in OPTS else dd[:], t + 1, first=True, last=False)

        ncv = work.tile([H, BL], b16 if split else f32, name="ncv", tag="ncv")
        nc.vector.tensor_tensor(ncv[:], nn[:], cc[:], OP.mult)

        if split and t + 1 < T_:
            pass  # ncv's matmuls are emitted at the top of step t+1

        # h = (1-z)n + z h, for the running sum and next step's dd.
        heng.tensor_tensor(h[:], ncv[:], dd[:], OP.add)
        heng.tensor_tensor(hs[:], hs[:], h[:], OP.add)

        ncv_p = ncv

    out_sb = const.tile([H, BL], f32, name="out_sb")
    nc.scalar.mul(out_sb[:], hs[:], 1.0 / T_)
    # Device-side AllGather so every core's yT holds all 8 cores' results;
    # the host then fetches a single shard (1 RPC instead of 8).
    dram = ctx.enter_context(tc.tile_pool(name="ydram", bufs=1, space="DRAM"))
    y_in = dram.tile([H, BL], f32, name="y_in")
    y_out = dram.tile([NCORES * H, BL], f32, name="y_out")
    nc.sync.dma_start(y_in[:], out_sb[:])
    nc.gpsimd.collective_compute(
        "AllGather",
        mybir.AluOpType.bypass,
        replica_groups=[list(range(NCORES))],
        ins=[y_in.opt()],
        outs=[y_out.opt()],
    )
    nc.sync.dma_start(yT, y_out[:])


def build_program(T_=T, repeat=1):
    key = (T_, repeat, OPTS)
    if key in _PROG_CACHE:
        return _PROG_CACHE[key]
    import concourse.tile as tile
    from concourse import bacc, mybir

    f32, b16 = mybir.dt.float32, mybir.dt.bfloat16
    fp8 = mybir.dt.float8e4
    nc = bacc.Bacc(
        "TRN2", target_bir_lowering=False, debug=False, num_devices=NCORES
    )
    nchunk = (T_ + CHUNK - 1) // CHUNK
    xTs = [
        nc.dram_tensor(
            f"xT{c}",
            [I, min(CHUNK, T_ - c * CHUNK), BL],
            fp8,
            kind="ExternalInput",
        ).ap()
        for c in range(nchunk)
    ]
    wih = nc.dram_tensor("wih", [I, 3 * H], b16, kind="ExternalInput").ap()
    wb = nc.dram_tensor("wb", [1, 3 * H], b16, kind="ExternalInput").ap()
    bn = nc.dram_tensor("bn", [H, 1], f32, kind="ExternalInput").ap()
    whh = nc.dram_tensor("whh", [H, 3 * H], b16, kind="ExternalInput").ap()
    yT = nc.dram_tensor("yT", [NCORES * H, BL], f32, kind="ExternalOutput").ap()

    with tile.TileContext(nc) as tc:
        with ExitStack() as ctx:
            _emit(ctx, tc, nc, xTs, wih, wb, bn, whh, yT, T_, repeat)
    nc.compile()
    _PROG_CACHE[key] = nc
    return nc


def _prep_weights(W_ih, W_hh, b_ih, b_hh):
    W_ih = np.asarray(W_ih, dtype=np.float32)
    W_hh = np.asarray(W_hh, dtype=np.float32)
    b_ih = np.asarray(b_ih, dtype=np.float32)
    b_hh = np.asarray(b_hh, dtype=np.float32)

    # Bias row, applied in PSUM via a K=1 matmul against an on-device ones
    # row: (b_r_tot, b_z_tot, b_ih_n).  b_hh_n is applied inside the
    # r-multiply (bn).
    wbr = np.empty((1, 3 * H), np.float32)
    wbr[0, 0:H] = b_ih[0:H] + b_hh[0:H]
    wbr[0, H : 2 * H] = b_ih[H : 2 * H] + b_hh[H : 2 * H]
    wbr[0, 2 * H :] = b_ih[2 * H :]
    wbr = wbr.astype(bf16)                             # [1, 3H]
    wihT = np.ascontiguousarray(W_ih.T).astype(bf16)   # [I, 3H]
    whhT = np.ascontiguousarray(W_hh.T).astype(bf16)   # [H, 3H]
    bnv = b_hh[2 * H :].reshape(H, 1).astype(np.float32)
    return wihT, wbr, whhT, bnv


try:
    import torch as _torch

    _TORCH_F8 = _torch.float8_e4m3fn  # bit-identical to ml_dtypes.float8_e4m3
except Exception:
    _torch = None


def _prep_x_chunk(x, c, T_=T):
    # [B, T, I] f32 -> chunk c transposed fp8 (no ones row; filled on-device):
    # (NC, I, steps, BL) where element (n, i, t, b) = x[n*BL+b, c*CHUNK+t, i]
    steps = min(CHUNK, T_ - c * CHUNK)
    if _torch is not None:
        t = _torch.from_numpy(x).reshape(NCORES, BL, x.shape[1], I)[
            :, :, c * CHUNK : c * CHUNK + steps
        ].permute(0, 3, 2, 1)
        out = _torch.empty((NCORES, I, steps, BL), dtype=_TORCH_F8)
        out.copy_(t)
        return out.view(_torch.uint8).numpy().view(f8)
    xs = x.reshape(NCORES, BL, x.shape[1], I)[:, :, c * CHUNK : c * CHUNK + steps]
    return xs.transpose(0, 3, 2, 1).astype(f8)


def make_in_maps(x, W_ih, W_hh, b_ih, b_hh, T_=T):
    # Per-core input dicts (used by the traced bass_utils path).
    wihT, wbr, whhT, bnv = _prep_weights(W_ih, W_hh, b_ih, b_hh)
    x = np.asarray(x, dtype=np.float32)
    nchunk = (T_ + CHUNK - 1) // CHUNK
    chunks = [_prep_x_chunk(x, c, T_) for c in range(nchunk)]
    return [
        {
            **{f"xT{k}": chunks[k][c] for k in range(nchunk)},
            "wih": wihT,
            "wb": wbr,
            "whh": whhT,
            "bn": bnv,
        }
        for c in range(NCORES)
    ]


class _Runner:
    """Caches the built Bass program and the jitted sharded executable.

    run_bass_kernel_spmd constructs a fresh jax.jit(shard_map(...)) closure
    per call, which re-traces, re-lowers and re-invokes the NEFF compile
    hook every time (~5.5s/call measured). Building it once here makes the
    warm path pure transfer+execute.
    """

    def __init__(self, T_=T):
        import jax
        from jax.sharding import Mesh, PartitionSpec
        from jax.experimental.shard_map import shard_map as shard_map_fn
        from concourse import mybir
        from concourse.bass2jax import (
            _bass_exec_p,
            install_neuronx_cc_hook,
            partition_id_tensor,
        )

        # Persistent XLA executable cache: a fresh process reuses the
        # compiled (NEFF-embedding) executable instead of re-running the
        # ~40s neuronx compile.
        try:
            jax.config.update("jax_compilation_cache_dir", "/tmp/jax_exe_cache")
            jax.config.update("jax_persistent_cache_min_compile_time_secs", 0.0)
            jax.config.update("jax_persistent_cache_min_entry_size_bytes", 0)
        except Exception:
            pass

        install_neuronx_cc_hook()
        nc = build_program(T_)
        self.nc = nc
        self.T_ = T_

        partition_name = (
            nc.partition_id_tensor.name if nc.partition_id_tensor else None
        )
        in_names, out_names, out_avals = [], [], []
        for alloc in nc.m.functions[0].allocations:
            if not isinstance(alloc, mybir.MemoryLocationSet):
                continue
            name = alloc.memorylocations[0].name
            if alloc.kind == "ExternalInput":
                if name != partition_name:
                    in_names.append(name)
            elif alloc.kind == "ExternalOutput":
                shape = tuple(alloc.tensor_shape)
                dtype = mybir.dt.np(alloc.dtype)
                out_names.append(name)
                out_avals.append(jax.core.ShapedArray(shape, dtype))
        self.in_names = in_names
        self.out_names = out_names
        self.out_avals = out_avals
        n_params = len(in_names)
        n_outs = len(out_avals)
        in_names_all = in_names + out_names + (
            [partition_name] if partition_name else []
        )
        donate = tuple(range(n_params, n_params + n_outs))

        def _body(*args):
            operands = list(args)
            if partition_name:
                operands.append(partition_id_tensor())
            outs = _bass_exec_p.bind(
                *operands,
                out_avals=tuple(out_avals),
                in_names=tuple(in_names_all),
                out_names=tuple(out_names),
                lowering_input_output_aliases=(),
                sim_require_finite=True,
                sim_require_nnan=True,
                nc=nc,
            )
            return tuple(outs)

        from jax.sharding import NamedSharding

        devices = jax.devices()[:NCORES]
        assert len(devices) == NCORES
        mesh = Mesh(np.asarray(devices), ("core",))
        self.x_sharding = NamedSharding(mesh, PartitionSpec("core"))
        self._device_put = jax.device_put
        self.sharded = jax.jit(
            shard_map_fn(
                _body,
                mesh=mesh,
                in_specs=(PartitionSpec("core"),) * (n_params + n_outs),
                out_specs=(PartitionSpec("core"),) * n_outs,
                check_rep=False,
            ),
            donate_argnums=donate,
            keep_unused=True,
        )
        self.nchunk = (T_ + CHUNK - 1) // CHUNK
        self._last_out = None  # previous call's output buffers, re-donated

    def __call__(self, x, W_ih, W_hh, b_ih, b_hh):
        # Ship the (tiny) weights first, then x chunk by chunk: device_put
        # is async, so the weight transfer and chunk k's host-side
        # transpose+fp8 cast overlap chunk k-1's transfer.
        x = np.asarray(x, dtype=np.float32)
        wihT, wbr, whhT, bnv = _prep_weights(W_ih, W_hh, b_ih, b_hh)
        by_name = {}
        for name, arr in (
            ("wih", wihT),
            ("wb", wbr),
            ("whh", whhT),
            ("bn", bnv),
        ):
            g = np.ascontiguousarray(
                np.broadcast_to(arr, (NCORES, *arr.shape))
            ).reshape(NCORES * arr.shape[0], *arr.shape[1:])
            by_name[name] = self._device_put(g, self.x_sharding)
        for c in range(self.nchunk):
            xc = _prep_x_chunk(x, c, self.T_)
            by_name[f"xT{c}"] = self._device_put(
                xc.reshape(NCORES * I, xc.shape[2], BL), self.x_sharding
            )
        concat_in = [by_name[n] for n in self.in_names]
        # The kernel writes every output element, so the donated output
        # buffers' contents are irrelevant; re-donate the previous call's
        # device-resident outputs to skip the host->device zero transfer.
        # The first call device_puts zeros so every call presents the same
        # committed-array signature (a numpy-vs-device mismatch would
        # trigger a second executable compile).
        if self._last_out is not None:
            donation = self._last_out
        else:
            donation = [
                self._device_put(
                    np.zeros((NCORES * a.shape[0], *a.shape[1:]), a.dtype),
                    self.x_sharding,
                )
                for a in self.out_avals
            ]
        out = self.sharded(*concat_in, *donation)
        self._last_out = list(out)
        # yT is allgathered on-device: every core's output holds all 8
        # cores' results -> fetch a single shard (1 RPC instead of 8).
        yT_arr = out[self.out_names.index("yT")]
        shard0 = min(yT_arr.addressable_shards, key=lambda s: s.index[0].start or 0)
        yT = np.asarray(shard0.data)
        y = yT.reshape(NCORES, H, BL).transpose(0, 2, 1).reshape(B, H)
        return np.ascontiguousarray(y, dtype=np.float32)


_RUNNER = None


def _get_runner():
    global _RUNNER
    if _RUNNER is None:
        _RUNNER = _Runner()
    return _RUNNER


def run(x, W_ih, W_hh, b_ih, b_hh, T_=T, trace=False, **kw):
    if trace:
        # Traced path (NTFF profile) via the generic SPMD runner; raises in
        # environments without the axon NTFF hook.
        from concourse import bass_utils

        nc = build_program(T_)
        in_maps = make_in_maps(x, W_ih, W_hh, b_ih, b_hh, T_)
        res = bass_utils.run_bass_kernel_spmd(
            nc, in_maps, core_ids=list(range(NCORES)), trace=True, **kw
        )
        yT = np.asarray(res.results[0]["yT"], dtype=np.float32)
        y = yT.reshape(NCORES, H, BL).transpose(0, 2, 1).reshape(B, H)
        return y, res

    runner = _get_runner() if T_ == T else _Runner(T_)
    y = runner(x, W_ih, W_hh, b_ih, b_hh)

    class _Res:
        exec_time_ns = None
        results = None

    return y, _Res()


def kernel(**inputs) -> np.ndarray:
    runner = _get_runner()
    return runner(
        inputs["x"], inputs["W_ih"], inputs["W_hh"], inputs["b_ih"], inputs["b_hh"]
    )


# revision 37
# speedup vs baseline: 1.2516x; 1.0090x over previous
"""AudioGRU Trainium2 Bass kernel.

Single-layer GRU (PyTorch gate order r,z,n) over T=2000 steps followed by a
mean over time. Data-parallel over the batch axis across 8 NeuronCores
(B=256 -> 32 per core); weights replicated; the time recurrence is local.

Device kernel: everything lives transposed on-chip, [H=128 partitions,
batch free]. Per step the gate pre-activations gh = W_hh @ h accumulate into
PSUM on top of the input projections gx = W_ih @ x_t (computed by a block
"sweep" matmul 16 steps ahead using PE idle time); the r/z/n biases land in
the same PSUM banks via a K=1 matmul of the bias row against a const ones
tile. r and z live in one 2-bank PSUM tensor so a single fused sigmoid
covers both. With split_mm the recurrence matmul is decomposed as
W @ h = W @ (ncv + dd) with dd = z*h_prev and ncv = (1-z)*n, so the h-update
add leaves the critical cycle. The state h is bf16; x ships as fp8_e4m3
(W_ih stays bf16 — mixed non-fp32 matmul dtypes are allowed), which halves
the host->device transfer; measured end-to-end rel err ~5e-3 vs the fp32
reference, within the 2e-2 gate. A trailing device-side AllGather replicates
the [H, BL] result across cores so the host fetches one shard.

Host path (the wall-clock bottleneck — the tunnel to the device moves
~50 MB/s): the jax/PJRT executable is built once and cached at module level
(the generic run_bass_kernel_spmd re-traces and re-lowers on every call,
which costs ~5.5s/call). x is cast/transposed per T-chunk with torch (4x
faster than ml_dtypes) and each chunk is device_put asynchronously so prep
overlaps transfer; output buffers from the previous call are re-donated to
skip the host->device zero-init transfer.
"""

import os
import sys
import numpy as np
import ml_dtypes
from contextlib import ExitStack

for _p in ("/opt/trn_rl_repo", "/root/.axon_site/_ro/trn_rl_repo"):
    if os.path.isdir(_p) and _p not in sys.path:
        sys.path.insert(0, _p)



B, T, I, H = 256, 2000, 23, 128
NCORES = 8
BL = B // NCORES          # 32 batch per core
BLK = 16                  # psum block: 16 steps * 32 batch = 512 f32 = one bank
CHUNK = 400               # x DMA chunk + host transfer chunk, in timesteps
assert CHUNK % BLK == 0
bf16 = ml_dtypes.bfloat16
f8 = ml_dtypes.float8_e4m3

_PROG_CACHE = {}
OPTS = frozenset(("split_mm", "hsum_pool"))


def _emit(ctx, tc, nc, xTs, wih, wb, bn, whh, yT, T_, repeat=1):
    from concourse import mybir

    f32, b16 = mybir.dt.float32, mybir.dt.bfloat16
    fp8 = mybir.dt.float8e4
    AF = mybir.ActivationFunctionType
    OP = mybir.AluOpType
    NBLK = T_ // BLK
    nchunk = (T_ + CHUNK - 1) // CHUNK
    split = "split_mm" in OPTS
    heng = nc.gpsimd if "hsum_pool" in OPTS else nc.vector

    const = ctx.enter_context(tc.tile_pool(name="const", bufs=1))
    xpool = ctx.enter_context(tc.tile_pool(name="xp", bufs=3))
    gxp_rz = ctx.enter_context(tc.tile_pool(name="gxrz", bufs=2, space="PSUM"))
    gxp_n = ctx.enter_context(tc.tile_pool(name="gxn", bufs=2, space="PSUM"))
    ghp = ctx.enter_context(tc.tile_pool(name="ghp", bufs=1, space="PSUM"))
    work = ctx.enter_context(tc.tile_pool(name="wk", bufs=3))

    # wih/whh arrive sharded (1/8th per core) and are allgathered on-device
    # -- the host ships each matrix once instead of 8 replicated copies.
    wdram = ctx.enter_context(tc.tile_pool(name="wdram", bufs=1, space="DRAM"))
    wih_bi = wdram.tile([(I + 1) // NCORES, 3 * H], b16, name="wih_bi")
    nc.sync.dma_start(wih_bi[:], wih)
    wih_full = wdram.tile([I + 1, 3 * H], b16, name="wih_full")
    nc.gpsimd.collective_compute(
        "AllGather",
        OP.bypass,
        replica_groups=[list(range(NCORES))],
        ins=[wih_bi.opt()],
        outs=[wih_full.opt()],
    )
    whh_bi = wdram.tile([H // NCORES, 3 * H], b16, name="whh_bi")
    nc.sync.dma_start(whh_bi[:], whh)
    whh_full = wdram.tile([H, 3 * H], b16, name="whh_full")
    nc.gpsimd.collective_compute(
        "AllGather",
        OP.bypass,
        replica_groups=[list(range(NCORES))],
        ins=[whh_bi.opt()],
        outs=[whh_full.opt()],
    )
    wih_sb = const.tile([I, 3 * H], b16, name="wih_sb")
    nc.sync.dma_start(wih_sb[:], wih_full[0:I, :])
    wb_sb = const.tile([1, 3 * H], b16, name="wb_sb")
    nc.sync.dma_start(wb_sb[:], wb)
    whh_sb = const.tile([H, 3 * H], b16, name="whh_sb")
    nc.sync.dma_start(whh_sb[:], whh_full[:])
    bn_sb = const.tile([H, 1], f32, name="bn_sb")
    nc.sync.dma_start(bn_sb[:], bn)
    # K=1 matmuls against this ones row add the (r,z,n) biases into PSUM,
    # replacing the augmented ones-row that used to ship with x.
    ones_sb = const.tile([1, BLK * BL], b16, name="ones_sb")
    nc.vector.memset(ones_sb[:], 1.0)

    h = const.tile([H, BL], b16, name="h_state")
    hs = const.tile([H, BL], f32, name="h_sum")

    if repeat > 1:
        ctx.enter_context(tc.For_i(0, repeat, 1, name="rep"))
    if not split:
        nc.vector.memset(h[:], 0.0)
    nc.vector.memset(hs[:], 0.0)

    xs = []

    def load_chunk(c):
        steps = min(CHUNK, T_ - c * CHUNK)
        xc = xpool.tile([I, steps * BL], fp8, name="xc", tag="xc")
        nc.sync.dma_start(xc[:], xTs[c][:, :, :])
        return xc

    xs.append(load_chunk(0))
    if nchunk > 1:
        xs.append(load_chunk(1))

    # gh_n scratch bank: two rotating [H, BL] slots
    GHW = 512 if "ghn_2bank" in OPTS else BL
    ghn = ghp.tile([H, 1024 if "ghn_2bank" in OPTS else 512], f32, name="ghn_bank")

    def alloc_block():
        # r and z share one 2-bank tensor: cols 0..512 = r, 512..1024 = z.
        grz = gxp_rz.tile([H, 2 * BLK * BL], f32, name="grz", tag="grz")
        gn = gxp_n.tile([H, BLK * BL], f32, name="gn", tag="gn")
        gns = None
        if "gxn_sbuf" in OPTS:
            gns = work.tile([H, BLK * BL], f32, name="gns", tag="gns", bufs=2)
        return (grz, gn, gns)

    def sweep_block(blk, b):
        # Input projections plus biases (K=1 matmul on the ones row) for
        # block b.
        t0 = b * BLK
        c, o = divmod(t0, CHUNK)
        rhs = xs[c][:, o * BL : (o + BLK) * BL]
        grz, gn, gns = blk
        for g, out in ((0, grz[:, : BLK * BL]), (1, grz[:, BLK * BL :]), (2, gn[:, :])):
            nc.tensor.matmul(
                out,
                wih_sb[:, g * H : (g + 1) * H],
                rhs,
                start=True,
                stop=False,
                skip_group_check=True,
            )
            nc.tensor.matmul(
                out,
                wb_sb[:, g * H : (g + 1) * H],
                ones_sb[:],
                start=False,
                stop=(g == 2),
                skip_group_check=True,
            )
        if gns is not None:
            nc.scalar.copy(gns[:], gn[:])

    blocks = [None, None]

    def rec_mms(vec, t_target, first, last):
        # Accumulate W_g @ vec into step t_target's gate psum slices.
        bt, jt = divmod(t_target, BLK)
        grz, gn = blocks[bt % 2][:2]
        slt = (t_target % 2) * GHW
        blk_last = last and jt == BLK - 1
        nc.tensor.matmul(
            grz[:, jt * BL : (jt + 1) * BL],
            whh_sb[:, 0:H], vec, start=False, stop=blk_last,
            skip_group_check=True,
        )
        nc.tensor.matmul(
            grz[:, BLK * BL + jt * BL : BLK * BL + (jt + 1) * BL],
            whh_sb[:, H : 2 * H], vec, start=False, stop=blk_last,
            skip_group_check=True,
        )
        nc.tensor.matmul(
            ghn[:, slt : slt + BL],
            whh_sb[:, 2 * H : 3 * H], vec, start=first, stop=last,
            skip_group_check=True,
        )

    blocks[0] = alloc_block()
    sweep_block(blocks[0], 0)

    ncv_p = None  # previous step's ncv (split mode)
    for t in range(T_):
        b_, j = divmod(t, BLK)
        if j == 0:
            if t % CHUNK == 0 and t // CHUNK + 2 < nchunk:
                xs.append(load_chunk(t // CHUNK + 2))
            if b_ + 1 < NBLK:
                blocks[(b_ + 1) % 2] = alloc_block()
                sweep_block(blocks[(b_ + 1) % 2], b_ + 1)

        grz, gn, gns = blocks[b_ % 2]
        sl = slice(j * BL, (j + 1) * BL)
        slz = slice(BLK * BL + j * BL, BLK * BL + (j + 1) * BL)
        slot = (t % 2) * GHW

        if "x_notdep" in OPTS:
            if t > 0:
                rec_mms(whh_sb[:, 0:BL], t, first=False, last=True)
        elif split:
            # Step t's gh accumulated from dd_{t-1} (emitted last step) plus
            # ncv_{t-1} here; nothing at t=0 (h_{-1} = 0).
            if ncv_p is not None:
                rec_mms(ncv_p[:], t, first=False, last=True)
        else:
            rec_mms(h[:], t, first=True, last=True)

        # Fused sigmoid over r|z (biases already in psum), then 1-z as a
        # sigmoid with scale=-1, placed between so tanh isn't delayed.
        rz = work.tile([H, 2, BL], f32, name="rz", tag="rz")
        if "unfuse_sig" in OPTS:
            nc.scalar.activation(rz[:, 0, :], grz[:, sl], AF.Sigmoid)
            nc.scalar.activation(rz[:, 1, :], grz[:, slz], AF.Sigmoid)
        else:
            nc.scalar.activation(
                rz[:],
                grz.rearrange("p (g c) -> p g c", g=2)[:, :, j * BL : (j + 1) * BL],
                AF.Sigmoid,
            )
        cc = work.tile([H, BL], f32, name="cc", tag="cc")
        if "cc_pool" in OPTS:
            heng.tensor_scalar(cc[:], rz[:, 1, :], -1.0, 1.0, OP.mult, OP.add)
        elif "cc_dve" in OPTS:
            nc.vector.tensor_scalar(cc[:], rz[:, 1, :], -1.0, 1.0, OP.mult, OP.add)
        else:
            nc.scalar.activation(cc[:], grz[:, slz], AF.Sigmoid, scale=-1.0)

        # t1 = (gh_n + b_hh_n) * r ; t2 = t1 + gx_n ; n = tanh(t2)
        t1 = work.tile([H, BL], f32, name="t1", tag="t1")
        if split and t == 0:
            nc.vector.tensor_scalar(t1[:], rz[:, 0, :], bn_sb[:, 0:1], None, OP.mult)
        else:
            nc.vector.scalar_tensor_tensor(
                t1[:], ghn[:, slot : slot + BL], bn_sb[:, 0:1], rz[:, 0, :],
                OP.add, OP.mult,
            )
        t2 = work.tile([H, BL], f32, name="t2", tag="t2")
        gn_src = gns if gns is not None else gn
        nc.vector.tensor_tensor(t2[:], t1[:], gn_src[:, sl], OP.add)
        nn = work.tile([H, BL], f32, name="nn", tag="nn")
        nc.scalar.activation(nn[:], t1[:] if "x_not2" in OPTS else t2[:], AF.Tanh)

        dd = work.tile([H, BL], b16 if split else f32, name="dd", tag="dd")
        if split and t == 0:
            nc.vector.tensor_scalar(dd[:], rz[:, 1, :], 0.0, None, OP.mult)
        else:
            nc.vector.tensor_tensor(dd[:], rz[:, 1, :], h[:], OP.mult)

        if split and t + 1 < T_:
            # dd's matmuls fire before tanh completes.
            rec_mms(whh_sb[:, 0:BL] if "x_notdep" in OPTS else dd[:], t + 1, first=True, last=False)

        ncv = work.tile([H, BL], b16 if split else f32, name="ncv", tag="ncv")
        nc.vector.tensor_tensor(ncv[:], nn[:], cc[:], OP.mult)

        if split and t + 1 < T_:
            pass  # ncv's matmuls are emitted at the top of step t+1

        # h = (1-z)n + z h, for the running sum and next step's dd.
        heng.tensor_tensor(h[:], ncv[:], dd[:], OP.add)
        heng.tensor_tensor(hs[:], hs[:], h[:], OP.add)

        ncv_p = ncv

    out_sb = const.tile([H, BL], f32, name="out_sb")
    nc.scalar.mul(out_sb[:], hs[:], 1.0 / T_)
    # Device-side AllGather so every core's yT holds all 8 cores' results;
    # the host then fetches a single shard (1 RPC instead of 8).
    dram = ctx.enter_context(tc.tile_pool(name="ydram", bufs=1, space="DRAM"))
    y_in = dram.tile([H, BL], f32, name="y_in")
    y_out = dram.tile([NCORES * H, BL], f32, name="y_out")
    nc.sync.dma_start(y_in[:], out_sb[:])
    nc.gpsimd.collective_compute(
        "AllGather",
        mybir.AluOpType.bypass,
        replica_groups=[list(range(NCORES))],
        ins=[y_in.opt()],
        outs=[y_out.opt()],
    )
    nc.sync.dma_start(yT, y_out[:])


def build_program(T_=T, repeat=1):
    key = (T_, repeat, OPTS)
    if key in _PROG_CACHE:
        return _PROG_CACHE[key]
    import concourse.tile as tile
    from concourse import bacc, mybir

    f32, b16 = mybir.dt.float32, mybir.dt.bfloat16
    fp8 = mybir.dt.float8e4
    nc = bacc.Bacc(
        "TRN2", target_bir_lowering=False, debug=False, num_devices=NCORES
    )
    nchunk = (T_ + CHUNK - 1) // CHUNK
    xTs = [
        nc.dram_tensor(
            f"xT{c}",
            [I, min(CHUNK, T_ - c * CHUNK), BL],
            fp8,
            kind="ExternalInput",
        ).ap()
        for c in range(nchunk)
    ]
    wih = nc.dram_tensor(
        "wih", [(I + 1) // NCORES, 3 * H], b16, kind="ExternalInput"
    ).ap()
    wb = nc.dram_tensor("wb", [1, 3 * H], b16, kind="ExternalInput").ap()
    bn = nc.dram_tensor("bn", [H, 1], f32, kind="ExternalInput").ap()
    whh = nc.dram_tensor(
        "whh", [H // NCORES, 3 * H], b16, kind="ExternalInput"
    ).ap()
    yT = nc.dram_tensor("yT", [NCORES * H, BL], f32, kind="ExternalOutput").ap()

    with tile.TileContext(nc) as tc:
        with ExitStack() as ctx:
            _emit(ctx, tc, nc, xTs, wih, wb, bn, whh, yT, T_, repeat)
    nc.compile()
    _PROG_CACHE[key] = nc
    return nc


def _prep_weights(W_ih, W_hh, b_ih, b_hh):
    W_ih = np.asarray(W_ih, dtype=np.float32)
    W_hh = np.asarray(W_hh, dtype=np.float32)
    b_ih = np.asarray(b_ih, dtype=np.float32)
    b_hh = np.asarray(b_hh, dtype=np.float32)

    # Bias row, applied in PSUM via a K=1 matmul against an on-device ones
    # row: (b_r_tot, b_z_tot, b_ih_n).  b_hh_n is applied inside the
    # r-multiply (bn).
    wbr = np.empty((1, 3 * H), np.float32)
    wbr[0, 0:H] = b_ih[0:H] + b_hh[0:H]
    wbr[0, H : 2 * H] = b_ih[H : 2 * H] + b_hh[H : 2 * H]
    wbr[0, 2 * H :] = b_ih[2 * H :]
    wbr = wbr.astype(bf16)                             # [1, 3H]
    # wih is padded to I+1 rows so it shards evenly across the 8 cores for
    # the on-device weight allgather (the pad row is dropped device-side).
    wihT = np.zeros((I + 1, 3 * H), np.float32)
    wihT[:I] = W_ih.T
    wihT = wihT.astype(bf16)                           # [I+1, 3H]
    whhT = np.ascontiguousarray(W_hh.T).astype(bf16)   # [H, 3H]
    bnv = b_hh[2 * H :].reshape(H, 1).astype(np.float32)
    return wihT, wbr, whhT, bnv


try:
    import torch as _torch

    _TORCH_F8 = _torch.float8_e4m3fn  # bit-identical to ml_dtypes.float8_e4m3
except Exception:
    _torch = None


def _prep_x_chunk(x, c, T_=T):
    # [B, T, I] f32 -> chunk c transposed fp8 (no ones row; filled on-device):
    # (NC, I, steps, BL) where element (n, i, t, b) = x[n*BL+b, c*CHUNK+t, i]
    steps = min(CHUNK, T_ - c * CHUNK)
    if _torch is not None:
        t = _torch.from_numpy(x).reshape(NCORES, BL, x.shape[1], I)[
            :, :, c * CHUNK : c * CHUNK + steps
        ].permute(0, 3, 2, 1)
        out = _torch.empty((NCORES, I, steps, BL), dtype=_TORCH_F8)
        out.copy_(t)
        return out.view(_torch.uint8).numpy().view(f8)
    xs = x.reshape(NCORES, BL, x.shape[1], I)[:, :, c * CHUNK : c * CHUNK + steps]
    return xs.transpose(0, 3, 2, 1).astype(f8)


def make_in_maps(x, W_ih, W_hh, b_ih, b_hh, T_=T):
    # Per-core input dicts (used by the traced bass_utils path).
    wihT, wbr, whhT, bnv = _prep_weights(W_ih, W_hh, b_ih, b_hh)
    x = np.asarray(x, dtype=np.float32)
    nchunk = (T_ + CHUNK - 1) // CHUNK
    chunks = [_prep_x_chunk(x, c, T_) for c in range(nchunk)]
    rih = (I + 1) // NCORES
    rhh = H // NCORES
    return [
        {
            **{f"xT{k}": chunks[k][c] for k in range(nchunk)},
            "wih": wihT[c * rih : (c + 1) * rih],
            "wb": wbr,
            "whh": whhT[c * rhh : (c + 1) * rhh],
            "bn": bnv,
        }
        for c in range(NCORES)
    ]


class _Runner:
    """Caches the built Bass program and the jitted sharded executable.

    run_bass_kernel_spmd constructs a fresh jax.jit(shard_map(...)) closure
    per call, which re-traces, re-lowers and re-invokes the NEFF compile
    hook every time (~5.5s/call measured). Building it once here makes the
    warm path pure transfer+execute.
    """

    def __init__(self, T_=T):
        import jax
        from jax.sharding import Mesh, PartitionSpec
        from jax.experimental.shard_map import shard_map as shard_map_fn
        from concourse import mybir
        from concourse.bass2jax import (
            _bass_exec_p,
            install_neuronx_cc_hook,
            partition_id_tensor,
        )

        # Persistent XLA executable cache: a fresh process reuses the
        # compiled (NEFF-embedding) executable instead of re-running the
        # ~40s neuronx compile.
        try:
            jax.config.update("jax_compilation_cache_dir", "/tmp/jax_exe_cache")
            jax.config.update("jax_persistent_cache_min_compile_time_secs", 0.0)
            jax.config.update("jax_persistent_cache_min_entry_size_bytes", 0)
        except Exception:
            pass

        install_neuronx_cc_hook()
        nc = build_program(T_)
        self.nc = nc
        self.T_ = T_

        partition_name = (
            nc.partition_id_tensor.name if nc.partition_id_tensor else None
        )
        in_names, out_names, out_avals = [], [], []
        for alloc in nc.m.functions[0].allocations:
            if not isinstance(alloc, mybir.MemoryLocationSet):
                continue
            name = alloc.memorylocations[0].name
            if alloc.kind == "ExternalInput":
                if name != partition_name:
                    in_names.append(name)
            elif alloc.kind == "ExternalOutput":
                shape = tuple(alloc.tensor_shape)
                dtype = mybir.dt.np(alloc.dtype)
                out_names.append(name)
                out_avals.append(jax.core.ShapedArray(shape, dtype))
        self.in_names = in_names
        self.out_names = out_names
        self.out_avals = out_avals
        n_params = len(in_names)
        n_outs = len(out_avals)
        in_names_all = in_names + out_names + (
            [partition_name] if partition_name else []
        )
        donate = tuple(range(n_params, n_params + n_outs))

        def _body(*args):
            operands = list(args)
            if partition_name:
                operands.append(partition_id_tensor())
            outs = _bass_exec_p.bind(
                *operands,
                out_avals=tuple(out_avals),
                in_names=tuple(in_names_all),
                out_names=tuple(out_names),
                lowering_input_output_aliases=(),
                sim_require_finite=True,
                sim_require_nnan=True,
                nc=nc,
            )
            return tuple(outs)

        from jax.sharding import NamedSharding

        devices = jax.devices()[:NCORES]
        assert len(devices) == NCORES
        mesh = Mesh(np.asarray(devices), ("core",))
        self.x_sharding = NamedSharding(mesh, PartitionSpec("core"))
        self._device_put = jax.device_put
        self.sharded = jax.jit(
            shard_map_fn(
                _body,
                mesh=mesh,
                in_specs=(PartitionSpec("core"),) * (n_params + n_outs),
                out_specs=(PartitionSpec("core"),) * n_outs,
                check_rep=False,
            ),
            donate_argnums=donate,
            keep_unused=True,
        )
        self.nchunk = (T_ + CHUNK - 1) // CHUNK
        self._last_out = None  # previous call's output buffers, re-donated

    def __call__(self, x, W_ih, W_hh, b_ih, b_hh):
        # Ship the (tiny) weights first, then x chunk by chunk: device_put
        # is async, so the weight transfer and chunk k's host-side
        # transpose+fp8 cast overlap chunk k-1's transfer.
        x = np.asarray(x, dtype=np.float32)
        wihT, wbr, whhT, bnv = _prep_weights(W_ih, W_hh, b_ih, b_hh)
        by_name = {}
        # wih/whh ship sharded (each core gets 1/8th; the device allgathers);
        # wb/bn are tiny and ship replicated.
        by_name["wih"] = self._device_put(wihT, self.x_sharding)
        by_name["whh"] = self._device_put(whhT, self.x_sharding)
        for name, arr in (("wb", wbr), ("bn", bnv)):
            g = np.ascontiguousarray(
                np.broadcast_to(arr, (NCORES, *arr.shape))
            ).reshape(NCORES * arr.shape[0], *arr.shape[1:])
            by_name[name] = self._device_put(g, self.x_sharding)
        for c in range(self.nchunk):
            xc = _prep_x_chunk(x, c, self.T_)
            by_name[f"xT{c}"] = self._device_put(
                xc.reshape(NCORES * I, xc.shape[2], BL), self.x_sharding
            )
        concat_in = [by_name[n] for n in self.in_names]
        # The kernel writes every output element, so the donated output
        # buffers' contents are irrelevant; re-donate the previous call's
        # device-resident outputs to skip the host->device zero transfer.
        # The first call device_puts zeros so every call presents the same
        # committed-array signature (a numpy-vs-device mismatch would
        # trigger a second executable compile).
        if self._last_out is not None:
            donation = self._last_out
        else:
            donation = [
                self._device_put(
                    np.zeros((NCORES * a.shape[0], *a.shape[1:]), a.dtype),
                    self.x_sharding,
                )
                for a in self.out_avals
            ]
        out = self.sharded(*concat_in, *donation)
        self._last_out = list(out)
        # yT is allgathered on-device: every core's output holds all 8
        # cores' results -> fetch a single shard (1 RPC instead of 8).
        yT_arr = out[self.out_names.index("yT")]
        shard0 = min(yT_arr.addressable_shards, key=lambda s: s.index[0].start or 0)
        yT = np.asarray(shard0.data)
        y = yT.reshape(NCORES, H, BL).transpose(0, 2, 1).reshape(B, H)
        return np.ascontiguousarray(y, dtype=np.float32)


_RUNNER = None


def _get_runner():
    global _RUNNER
    if _RUNNER is None:
        _RUNNER = _Runner()
    return _RUNNER


def run(x, W_ih, W_hh, b_ih, b_hh, T_=T, trace=False, **kw):
    if trace:
        # Traced path (NTFF profile) via the generic SPMD runner; raises in
        # environments without the axon NTFF hook.
        from concourse import bass_utils

        nc = build_program(T_)
        in_maps = make_in_maps(x, W_ih, W_hh, b_ih, b_hh, T_)
        res = bass_utils.run_bass_kernel_spmd(
            nc, in_maps, core_ids=list(range(NCORES)), trace=True, **kw
        )
        yT = np.asarray(res.results[0]["yT"], dtype=np.float32)
        y = yT.reshape(NCORES, H, BL).transpose(0, 2, 1).reshape(B, H)
        return y, res

    runner = _get_runner() if T_ == T else _Runner(T_)
    y = runner(x, W_ih, W_hh, b_ih, b_hh)

    class _Res:
        exec_time_ns = None
        results = None

    return y, _Res()


def kernel(**inputs) -> np.ndarray:
    runner = _get_runner()
    return runner(
        inputs["x"], inputs["W_ih"], inputs["W_hh"], inputs["b_ih"], inputs["b_hh"]
    )


# revision 44
# speedup vs baseline: 1.3121x; 1.0483x over previous
"""AudioGRU Trainium2 Bass kernel.

Single-layer GRU (PyTorch gate order r,z,n) over T=2000 steps followed by a
mean over time. Data-parallel over the batch axis across 8 NeuronCores
(B=256 -> 32 per core); weights replicated; the time recurrence is local.

Device kernel: everything lives transposed on-chip, [H=128 partitions,
batch free]. Per step the gate pre-activations gh = W_hh @ h accumulate into
PSUM on top of the input projections gx = W_ih @ x_t (computed by a block
"sweep" matmul 16 steps ahead using PE idle time); the r/z/n biases land in
the same PSUM banks via a K=1 matmul of the bias row against a const ones
tile. r and z live in one 2-bank PSUM tensor so a single fused sigmoid
covers both. With split_mm the recurrence matmul is decomposed as
W @ h = W @ (ncv + dd) with dd = z*h_prev and ncv = (1-z)*n, so the h-update
add leaves the critical cycle. The state h is bf16; x ships as fp8_e4m3
(W_ih stays bf16 — mixed non-fp32 matmul dtypes are allowed), which halves
the host->device transfer; measured end-to-end rel err ~5e-3 vs the fp32
reference, within the 2e-2 gate. A trailing device-side AllGather replicates
the [H, BL] result across cores so the host fetches one shard.

Host path (the wall-clock bottleneck — the tunnel to the device moves
~50 MB/s, with a flat ~73ms round-trip per RPC): the jax/PJRT executable is
built once and cached at module level (the generic run_bass_kernel_spmd
re-traces and re-lowers on every call, which costs ~5.5s/call). x is
cast/transposed with torch (4x faster than ml_dtypes) and shipped in a
single async device_put (per-put dispatch costs ~12ms, so one put beats
chunked ones); output buffers from the previous call are re-donated to skip
the host->device zero-init transfer.
"""

import os
import sys
import numpy as np
import ml_dtypes
from contextlib import ExitStack

for _p in ("/opt/trn_rl_repo", "/root/.axon_site/_ro/trn_rl_repo"):
    if os.path.isdir(_p) and _p not in sys.path:
        sys.path.insert(0, _p)



B, T, I, H = 256, 2000, 23, 128
NCORES = 8
BL = B // NCORES          # 32 batch per core
BLK = 16                  # psum block: 16 steps * 32 batch = 512 f32 = one bank
CHUNK = 400               # x DMA chunk + host transfer chunk, in timesteps
assert CHUNK % BLK == 0
bf16 = ml_dtypes.bfloat16
f8 = ml_dtypes.float8_e4m3

_PROG_CACHE = {}
OPTS = frozenset(("split_mm", "hsum_pool"))


def _emit(ctx, tc, nc, xTs, wih, wb, bn, whh, yT, T_, repeat=1):
    from concourse import mybir

    f32, b16 = mybir.dt.float32, mybir.dt.bfloat16
    fp8 = mybir.dt.float8e4
    AF = mybir.ActivationFunctionType
    OP = mybir.AluOpType
    NBLK = T_ // BLK
    nchunk = (T_ + CHUNK - 1) // CHUNK
    split = "split_mm" in OPTS
    heng = nc.gpsimd if "hsum_pool" in OPTS else nc.vector

    const = ctx.enter_context(tc.tile_pool(name="const", bufs=1))
    xpool = ctx.enter_context(tc.tile_pool(name="xp", bufs=3))
    gxp_rz = ctx.enter_context(tc.tile_pool(name="gxrz", bufs=2, space="PSUM"))
    gxp_n = ctx.enter_context(tc.tile_pool(name="gxn", bufs=2, space="PSUM"))
    ghp = ctx.enter_context(tc.tile_pool(name="ghp", bufs=1, space="PSUM"))
    work = ctx.enter_context(tc.tile_pool(name="wk", bufs=3))

    # wih/whh arrive sharded (1/8th per core) and are allgathered on-device
    # -- the host ships each matrix once instead of 8 replicated copies.
    wdram = ctx.enter_context(tc.tile_pool(name="wdram", bufs=1, space="DRAM"))
    wih_bi = wdram.tile([(I + 1) // NCORES, 3 * H], b16, name="wih_bi")
    nc.sync.dma_start(wih_bi[:], wih)
    wih_full = wdram.tile([I + 1, 3 * H], b16, name="wih_full")
    nc.gpsimd.collective_compute(
        "AllGather",
        OP.bypass,
        replica_groups=[list(range(NCORES))],
        ins=[wih_bi.opt()],
        outs=[wih_full.opt()],
    )
    whh_bi = wdram.tile([H // NCORES, 3 * H], b16, name="whh_bi")
    nc.sync.dma_start(whh_bi[:], whh)
    whh_full = wdram.tile([H, 3 * H], b16, name="whh_full")
    nc.gpsimd.collective_compute(
        "AllGather",
        OP.bypass,
        replica_groups=[list(range(NCORES))],
        ins=[whh_bi.opt()],
        outs=[whh_full.opt()],
    )
    wih_sb = const.tile([I, 3 * H], b16, name="wih_sb")
    nc.sync.dma_start(wih_sb[:], wih_full[0:I, :])
    wb_sb = const.tile([1, 3 * H], b16, name="wb_sb")
    nc.sync.dma_start(wb_sb[:], wb)
    whh_sb = const.tile([H, 3 * H], b16, name="whh_sb")
    nc.sync.dma_start(whh_sb[:], whh_full[:])
    bn_sb = const.tile([H, 1], f32, name="bn_sb")
    nc.sync.dma_start(bn_sb[:], bn)
    # K=1 matmuls against this ones row add the (r,z,n) biases into PSUM,
    # replacing the augmented ones-row that used to ship with x.
    ones_sb = const.tile([1, BLK * BL], b16, name="ones_sb")
    nc.vector.memset(ones_sb[:], 1.0)

    h = const.tile([H, BL], b16, name="h_state")
    hs = const.tile([H, BL], f32, name="h_sum")

    if repeat > 1:
        ctx.enter_context(tc.For_i(0, repeat, 1, name="rep"))
    if not split:
        nc.vector.memset(h[:], 0.0)
    nc.vector.memset(hs[:], 0.0)

    xs = []

    def load_chunk(c):
        steps = min(CHUNK, T_ - c * CHUNK)
        xc = xpool.tile([I, steps * BL], fp8, name="xc", tag="xc")
        nc.sync.dma_start(xc[:], xTs[:, c * CHUNK : c * CHUNK + steps, :])
        return xc

    xs.append(load_chunk(0))
    if nchunk > 1:
        xs.append(load_chunk(1))

    # gh_n scratch bank: two rotating [H, BL] slots
    GHW = 512 if "ghn_2bank" in OPTS else BL
    ghn = ghp.tile([H, 1024 if "ghn_2bank" in OPTS else 512], f32, name="ghn_bank")

    def alloc_block():
        # r and z share one 2-bank tensor: cols 0..512 = r, 512..1024 = z.
        grz = gxp_rz.tile([H, 2 * BLK * BL], f32, name="grz", tag="grz")
        gn = gxp_n.tile([H, BLK * BL], f32, name="gn", tag="gn")
        gns = None
        if "gxn_sbuf" in OPTS:
            gns = work.tile([H, BLK * BL], f32, name="gns", tag="gns", bufs=2)
        return (grz, gn, gns)

    def sweep_block(blk, b):
        # Input projections plus biases (K=1 matmul on the ones row) for
        # block b.
        t0 = b * BLK
        c, o = divmod(t0, CHUNK)
        rhs = xs[c][:, o * BL : (o + BLK) * BL]
        grz, gn, gns = blk
        for g, out in ((0, grz[:, : BLK * BL]), (1, grz[:, BLK * BL :]), (2, gn[:, :])):
            nc.tensor.matmul(
                out,
                wih_sb[:, g * H : (g + 1) * H],
                rhs,
                start=True,
                stop=False,
                skip_group_check=True,
            )
            nc.tensor.matmul(
                out,
                wb_sb[:, g * H : (g + 1) * H],
                ones_sb[:],
                start=False,
                stop=(g == 2),
                skip_group_check=True,
            )
        if gns is not None:
            nc.scalar.copy(gns[:], gn[:])

    blocks = [None, None]

    def rec_mms(vec, t_target, first, last):
        # Accumulate W_g @ vec into step t_target's gate psum slices.
        bt, jt = divmod(t_target, BLK)
        grz, gn = blocks[bt % 2][:2]
        slt = (t_target % 2) * GHW
        blk_last = last and jt == BLK - 1
        nc.tensor.matmul(
            grz[:, jt * BL : (jt + 1) * BL],
            whh_sb[:, 0:H], vec, start=False, stop=blk_last,
            skip_group_check=True,
        )
        nc.tensor.matmul(
            grz[:, BLK * BL + jt * BL : BLK * BL + (jt + 1) * BL],
            whh_sb[:, H : 2 * H], vec, start=False, stop=blk_last,
            skip_group_check=True,
        )
        nc.tensor.matmul(
            ghn[:, slt : slt + BL],
            whh_sb[:, 2 * H : 3 * H], vec, start=first, stop=last,
            skip_group_check=True,
        )

    blocks[0] = alloc_block()
    sweep_block(blocks[0], 0)

    ncv_p = None  # previous step's ncv (split mode)
    for t in range(T_):
        b_, j = divmod(t, BLK)
        if j == 0:
            if t % CHUNK == 0 and t // CHUNK + 2 < nchunk:
                xs.append(load_chunk(t // CHUNK + 2))
            if b_ + 1 < NBLK:
                blocks[(b_ + 1) % 2] = alloc_block()
                sweep_block(blocks[(b_ + 1) % 2], b_ + 1)

        grz, gn, gns = blocks[b_ % 2]
        sl = slice(j * BL, (j + 1) * BL)
        slz = slice(BLK * BL + j * BL, BLK * BL + (j + 1) * BL)
        slot = (t % 2) * GHW

        if "x_notdep" in OPTS:
            if t > 0:
                rec_mms(whh_sb[:, 0:BL], t, first=False, last=True)
        elif split:
            # Step t's gh accumulated from dd_{t-1} (emitted last step) plus
            # ncv_{t-1} here; nothing at t=0 (h_{-1} = 0).
            if ncv_p is not None:
                rec_mms(ncv_p[:], t, first=False, last=True)
        else:
            rec_mms(h[:], t, first=True, last=True)

        # Fused sigmoid over r|z (biases already in psum), then 1-z as a
        # sigmoid with scale=-1, placed between so tanh isn't delayed.
        rz = work.tile([H, 2, BL], f32, name="rz", tag="rz")
        if "unfuse_sig" in OPTS:
            nc.scalar.activation(rz[:, 0, :], grz[:, sl], AF.Sigmoid)
            nc.scalar.activation(rz[:, 1, :], grz[:, slz], AF.Sigmoid)
        else:
            nc.scalar.activation(
                rz[:],
                grz.rearrange("p (g c) -> p g c", g=2)[:, :, j * BL : (j + 1) * BL],
                AF.Sigmoid,
            )
        cc = work.tile([H, BL], f32, name="cc", tag="cc")
        if "cc_pool" in OPTS:
            heng.tensor_scalar(cc[:], rz[:, 1, :], -1.0, 1.0, OP.mult, OP.add)
        elif "cc_dve" in OPTS:
            nc.vector.tensor_scalar(cc[:], rz[:, 1, :], -1.0, 1.0, OP.mult, OP.add)
        else:
            nc.scalar.activation(cc[:], grz[:, slz], AF.Sigmoid, scale=-1.0)

        # t1 = (gh_n + b_hh_n) * r ; t2 = t1 + gx_n ; n = tanh(t2)
        t1 = work.tile([H, BL], f32, name="t1", tag="t1")
        if split and t == 0:
            nc.vector.tensor_scalar(t1[:], rz[:, 0, :], bn_sb[:, 0:1], None, OP.mult)
        else:
            nc.vector.scalar_tensor_tensor(
                t1[:], ghn[:, slot : slot + BL], bn_sb[:, 0:1], rz[:, 0, :],
                OP.add, OP.mult,
            )
        t2 = work.tile([H, BL], f32, name="t2", tag="t2")
        gn_src = gns if gns is not None else gn
        nc.vector.tensor_tensor(t2[:], t1[:], gn_src[:, sl], OP.add)
        nn = work.tile([H, BL], f32, name="nn", tag="nn")
        nc.scalar.activation(nn[:], t1[:] if "x_not2" in OPTS else t2[:], AF.Tanh)

        dd = work.tile([H, BL], b16 if split else f32, name="dd", tag="dd")
        if split and t == 0:
            nc.vector.tensor_scalar(dd[:], rz[:, 1, :], 0.0, None, OP.mult)
        else:
            nc.vector.tensor_tensor(dd[:], rz[:, 1, :], h[:], OP.mult)

        if split and t + 1 < T_:
            # dd's matmuls fire before tanh completes.
            rec_mms(whh_sb[:, 0:BL] if "x_notdep" in OPTS else dd[:], t + 1, first=True, last=False)

        ncv = work.tile([H, BL], b16 if split else f32, name="ncv", tag="ncv")
        nc.vector.tensor_tensor(ncv[:], nn[:], cc[:], OP.mult)

        if split and t + 1 < T_:
            pass  # ncv's matmuls are emitted at the top of step t+1

        # h = (1-z)n + z h, for the running sum and next step's dd.
        heng.tensor_tensor(h[:], ncv[:], dd[:], OP.add)
        heng.tensor_tensor(hs[:], hs[:], h[:], OP.add)

        ncv_p = ncv

    out_sb = const.tile([H, BL], f32, name="out_sb")
    nc.scalar.mul(out_sb[:], hs[:], 1.0 / T_)
    # Device-side AllGather so every core's yT holds all 8 cores' results;
    # the host then fetches a single shard (1 RPC instead of 8).
    dram = ctx.enter_context(tc.tile_pool(name="ydram", bufs=1, space="DRAM"))
    y_in = dram.tile([H, BL], f32, name="y_in")
    y_out = dram.tile([NCORES * H, BL], f32, name="y_out")
    nc.sync.dma_start(y_in[:], out_sb[:])
    nc.gpsimd.collective_compute(
        "AllGather",
        mybir.AluOpType.bypass,
        replica_groups=[list(range(NCORES))],
        ins=[y_in.opt()],
        outs=[y_out.opt()],
    )
    nc.sync.dma_start(yT, y_out[:])


def build_program(T_=T, repeat=1):
    key = (T_, repeat, OPTS)
    if key in _PROG_CACHE:
        return _PROG_CACHE[key]
    import concourse.tile as tile
    from concourse import bacc, mybir

    f32, b16 = mybir.dt.float32, mybir.dt.bfloat16
    fp8 = mybir.dt.float8e4
    nc = bacc.Bacc(
        "TRN2", target_bir_lowering=False, debug=False, num_devices=NCORES
    )
    xTs = nc.dram_tensor("xT", [I, T_, BL], fp8, kind="ExternalInput").ap()
    wih = nc.dram_tensor(
        "wih", [(I + 1) // NCORES, 3 * H], b16, kind="ExternalInput"
    ).ap()
    wb = nc.dram_tensor("wb", [1, 3 * H], b16, kind="ExternalInput").ap()
    bn = nc.dram_tensor("bn", [H, 1], f32, kind="ExternalInput").ap()
    whh = nc.dram_tensor(
        "whh", [H // NCORES, 3 * H], b16, kind="ExternalInput"
    ).ap()
    yT = nc.dram_tensor("yT", [NCORES * H, BL], f32, kind="ExternalOutput").ap()

    with tile.TileContext(nc) as tc:
        with ExitStack() as ctx:
            _emit(ctx, tc, nc, xTs, wih, wb, bn, whh, yT, T_, repeat)
    nc.compile()
    _PROG_CACHE[key] = nc
    return nc


def _prep_weights(W_ih, W_hh, b_ih, b_hh):
    W_ih = np.asarray(W_ih, dtype=np.float32)
    W_hh = np.asarray(W_hh, dtype=np.float32)
    b_ih = np.asarray(b_ih, dtype=np.float32)
    b_hh = np.asarray(b_hh, dtype=np.float32)

    # Bias row, applied in PSUM via a K=1 matmul against an on-device ones
    # row: (b_r_tot, b_z_tot, b_ih_n).  b_hh_n is applied inside the
    # r-multiply (bn).
    wbr = np.empty((1, 3 * H), np.float32)
    wbr[0, 0:H] = b_ih[0:H] + b_hh[0:H]
    wbr[0, H : 2 * H] = b_ih[H : 2 * H] + b_hh[H : 2 * H]
    wbr[0, 2 * H :] = b_ih[2 * H :]
    wbr = wbr.astype(bf16)                             # [1, 3H]
    # wih is padded to I+1 rows so it shards evenly across the 8 cores for
    # the on-device weight allgather (the pad row is dropped device-side).
    wihT = np.zeros((I + 1, 3 * H), np.float32)
    wihT[:I] = W_ih.T
    wihT = wihT.astype(bf16)                           # [I+1, 3H]
    whhT = np.ascontiguousarray(W_hh.T).astype(bf16)   # [H, 3H]
    bnv = b_hh[2 * H :].reshape(H, 1).astype(np.float32)
    return wihT, wbr, whhT, bnv


try:
    import torch as _torch

    _TORCH_F8 = _torch.float8_e4m3fn  # bit-identical to ml_dtypes.float8_e4m3
except Exception:
    _torch = None


def _prep_x(x, T_=T):
    # [B, T, I] f32 -> transposed fp8 (NC, I, T, BL) where element
    # (n, i, t, b) = x[n*BL+b, t, i]
    if _torch is not None:
        t = _torch.from_numpy(x).reshape(NCORES, BL, T_, I).permute(0, 3, 2, 1)
        out = _torch.empty((NCORES, I, T_, BL), dtype=_TORCH_F8)
        out.copy_(t)
        return out.view(_torch.uint8).numpy().view(f8)
    xs = x.reshape(NCORES, BL, T_, I)
    return xs.transpose(0, 3, 2, 1).astype(f8)


def make_in_maps(x, W_ih, W_hh, b_ih, b_hh, T_=T):
    # Per-core input dicts (used by the traced bass_utils path).
    wihT, wbr, whhT, bnv = _prep_weights(W_ih, W_hh, b_ih, b_hh)
    x = np.asarray(x, dtype=np.float32)
    xT_g = _prep_x(x, T_)
    rih = (I + 1) // NCORES
    rhh = H // NCORES
    return [
        {
            "xT": xT_g[c],
            "wih": wihT[c * rih : (c + 1) * rih],
            "wb": wbr,
            "whh": whhT[c * rhh : (c + 1) * rhh],
            "bn": bnv,
        }
        for c in range(NCORES)
    ]


class _Runner:
    """Caches the built Bass program and the jitted sharded executable.

    run_bass_kernel_spmd constructs a fresh jax.jit(shard_map(...)) closure
    per call, which re-traces, re-lowers and re-invokes the NEFF compile
    hook every time (~5.5s/call measured). Building it once here makes the
    warm path pure transfer+execute.
    """

    def __init__(self, T_=T):
        import jax
        from jax.sharding import Mesh, PartitionSpec
        from jax.experimental.shard_map import shard_map as shard_map_fn
        from concourse import mybir
        from concourse.bass2jax import (
            _bass_exec_p,
            install_neuronx_cc_hook,
            partition_id_tensor,
        )

        # Persistent XLA executable cache: a fresh process reuses the
        # compiled (NEFF-embedding) executable instead of re-running the
        # ~40s neuronx compile.
        try:
            jax.config.update("jax_compilation_cache_dir", "/tmp/jax_exe_cache")
            jax.config.update("jax_persistent_cache_min_compile_time_secs", 0.0)
            jax.config.update("jax_persistent_cache_min_entry_size_bytes", 0)
        except Exception:
            pass

        install_neuronx_cc_hook()
        nc = build_program(T_)
        self.nc = nc
        self.T_ = T_

        partition_name = (
            nc.partition_id_tensor.name if nc.partition_id_tensor else None
        )
        in_names, out_names, out_avals = [], [], []
        for alloc in nc.m.functions[0].allocations:
            if not isinstance(alloc, mybir.MemoryLocationSet):
                continue
            name = alloc.memorylocations[0].name
            if alloc.kind == "ExternalInput":
                if name != partition_name:
                    in_names.append(name)
            elif alloc.kind == "ExternalOutput":
                shape = tuple(alloc.tensor_shape)
                dtype = mybir.dt.np(alloc.dtype)
                out_names.append(name)
                out_avals.append(jax.core.ShapedArray(shape, dtype))
        self.in_names = in_names
        self.out_names = out_names
        self.out_avals = out_avals
        n_params = len(in_names)
        n_outs = len(out_avals)
        in_names_all = in_names + out_names + (
            [partition_name] if partition_name else []
        )
        donate = tuple(range(n_params, n_params + n_outs))

        def _body(*args):
            operands = list(args)
            if partition_name:
                operands.append(partition_id_tensor())
            outs = _bass_exec_p.bind(
                *operands,
                out_avals=tuple(out_avals),
                in_names=tuple(in_names_all),
                out_names=tuple(out_names),
                lowering_input_output_aliases=(),
                sim_require_finite=True,
                sim_require_nnan=True,
                nc=nc,
            )
            return tuple(outs)

        from jax.sharding import NamedSharding

        devices = jax.devices()[:NCORES]
        assert len(devices) == NCORES
        mesh = Mesh(np.asarray(devices), ("core",))
        self.x_sharding = NamedSharding(mesh, PartitionSpec("core"))
        self._device_put = jax.device_put
        self.sharded = jax.jit(
            shard_map_fn(
                _body,
                mesh=mesh,
                in_specs=(PartitionSpec("core"),) * (n_params + n_outs),
                out_specs=(PartitionSpec("core"),) * n_outs,
                check_rep=False,
            ),
            donate_argnums=donate,
            keep_unused=True,
        )
        self._last_out = None  # previous call's output buffers, re-donated

    def __call__(self, x, W_ih, W_hh, b_ih, b_hh):
        # Ship the (tiny) weights first, then x chunk by chunk: device_put
        # is async, so the weight transfer and chunk k's host-side
        # transpose+fp8 cast overlap chunk k-1's transfer.
        x = np.asarray(x, dtype=np.float32)
        wihT, wbr, whhT, bnv = _prep_weights(W_ih, W_hh, b_ih, b_hh)
        by_name = {}
        # wih/whh ship sharded (each core gets 1/8th; the device allgathers);
        # wb/bn are tiny and ship replicated.
        by_name["wih"] = self._device_put(wihT, self.x_sharding)
        by_name["whh"] = self._device_put(whhT, self.x_sharding)
        for name, arr in (("wb", wbr), ("bn", bnv)):
            g = np.ascontiguousarray(
                np.broadcast_to(arr, (NCORES, *arr.shape))
            ).reshape(NCORES * arr.shape[0], *arr.shape[1:])
            by_name[name] = self._device_put(g, self.x_sharding)
        # One put for all of x: per-put dispatch costs ~12ms (8 shard RPCs),
        # so one big put beats 5 chunked ones despite losing prep overlap.
        xT_g = _prep_x(x, self.T_)
        by_name["xT"] = self._device_put(
            xT_g.reshape(NCORES * I, self.T_, BL), self.x_sharding
        )
        concat_in = [by_name[n] for n in self.in_names]
        # The kernel writes every output element, so the donated output
        # buffers' contents are irrelevant; re-donate the previous call's
        # device-resident outputs to skip the host->device zero transfer.
        # The first call device_puts zeros so every call presents the same
        # committed-array signature (a numpy-vs-device mismatch would
        # trigger a second executable compile).
        if self._last_out is not None:
            donation = self._last_out
        else:
            donation = [
                self._device_put(
                    np.zeros((NCORES * a.shape[0], *a.shape[1:]), a.dtype),
                    self.x_sharding,
                )
                for a in self.out_avals
            ]
        out = self.sharded(*concat_in, *donation)
        self._last_out = list(out)
        # yT is allgathered on-device: every core's output holds all 8
        # cores' results -> fetch a single shard (1 RPC instead of 8).
        yT_arr = out[self.out_names.index("yT")]
        shard0 = min(yT_arr.addressable_shards, key=lambda s: s.index[0].start or 0)
        yT = np.asarray(shard0.data)
        y = yT.reshape(NCORES, H, BL).transpose(0, 2, 1).reshape(B, H)
        return np.ascontiguousarray(y, dtype=np.float32)


_RUNNER = None


def _get_runner():
    global _RUNNER
    if _RUNNER is None:
        _RUNNER = _Runner()
    return _RUNNER


def run(x, W_ih, W_hh, b_ih, b_hh, T_=T, trace=False, **kw):
    if trace:
        # Traced path (NTFF profile) via the generic SPMD runner; raises in
        # environments without the axon NTFF hook.
        from concourse import bass_utils

        nc = build_program(T_)
        in_maps = make_in_maps(x, W_ih, W_hh, b_ih, b_hh, T_)
        res = bass_utils.run_bass_kernel_spmd(
            nc, in_maps, core_ids=list(range(NCORES)), trace=True, **kw
        )
        yT = np.asarray(res.results[0]["yT"], dtype=np.float32)
        y = yT.reshape(NCORES, H, BL).transpose(0, 2, 1).reshape(B, H)
        return y, res

    runner = _get_runner() if T_ == T else _Runner(T_)
    y = runner(x, W_ih, W_hh, b_ih, b_hh)

    class _Res:
        exec_time_ns = None
        results = None

    return y, _Res()


def kernel(**inputs) -> np.ndarray:
    runner = _get_runner()
    return runner(
        inputs["x"], inputs["W_ih"], inputs["W_hh"], inputs["b_ih"], inputs["b_hh"]
    )
